# revision 2
# baseline (speedup 1.0000x reference)
"""Block-diagonal (per-frame) multi-head attention on 8 Trainium2 cores.

Problem: x[2,3200,512] -> QKV proj (H=8 heads, D=64) -> attention masked to
25-token frames (128 frames) -> out[2,3200,512].  N = 3200 = 128*25.

Sharding: 256 (batch, frame) groups; core c handles batch c//4, frames
(c%4)*32..+32  => 800 tokens/core, tiled as 8 x 100 tokens (4 frames).

Layout trick: host sends x pre-transposed (xT [512, 800]) so every matmul
contracts over the partition dim:
  qT/kT [feat, tok] = W.T @ xT   (lhsT = W slice, rhs = xT)
  v     [tok, feat] = xT.T @ Wv  (lhsT = xT slice, rhs = Wv)
Scores per (head, tile): S = qT_h.T @ kT_h and S^T = kT_h.T @ qT_h -- both
directly available, no transposes anywhere.  The -9e15 frame mask is rank-5
(ones + 4 frame indicators), injected by one small matmul that initializes
the PSUM accumulation group.  softmax skips max-subtraction (|scores| <~ 8).
PV uses E^T = exp(S^T) as the stationary operand with v natural as moving.
"""

import numpy as np

B, N, DIN = 2, 3200, 512
H, D = 8, 64
TL, JN = 128, 25
NCORES = 8
TOK = 800      # tokens per core
NT = 8         # token tiles per core
TT = 100       # tokens per tile (4 frames)
NEG = -9e15

# matmul dtype per stage: 'f32' | 'f32r' | 'bf16'
#   proj: QKV projection matmuls (and dtype of xT/W in SBUF+HBM)
#   qk:   dtype of qT/kT tiles (scores matmuls)
#   pv:   dtype of E^T and V tiles (PV matmul)
CONFIG = {"proj": "f32", "qk": "f32", "pv": "f32"}

_CACHE = {}
LAST_RESULT = None  # BassKernelResults of the most recent kernel() call


def _build(cfg, stage="full"):
    import concourse.bacc as bacc
    import concourse.tile as tile
    from concourse import mybir

    f32 = mybir.dt.float32
    bf16 = mybir.dt.bfloat16
    f16 = mybir.dt.float16
    f32r = mybir.dt.float32r
    AF = mybir.ActivationFunctionType
    ALU = mybir.AluOpType
    AX = mybir.AxisListType

    def io_dt(kind):
        return {"f32": f32, "f32r": f32r, "bf16": bf16, "f16": f16}[kind]

    def mm(ap, kind):
        return ap

    proj_dt, qk_dt, pv_dt = cfg["proj"], cfg["qk"], cfg["pv"]

    nc = bacc.Bacc("TRN2", target_bir_lowering=False, debug=False,
                   num_devices=NCORES)

    xt_d = nc.dram_tensor("xT", [DIN, TOK], io_dt(proj_dt),
                          kind="ExternalInput").ap()
    w_d = {}
    for nm in ("wq", "wk", "wv"):
        w_d[nm] = nc.dram_tensor(nm, [DIN, DIN], io_dt(proj_dt),
                                 kind="ExternalInput").ap()
    bqc_d = nc.dram_tensor("bqc", [128, 4], f32, kind="ExternalInput").ap()
    bkc_d = nc.dram_tensor("bkc", [128, 4], f32, kind="ExternalInput").ap()
    bvb_d = nc.dram_tensor("bvb", [128, DIN], f32, kind="ExternalInput").ap()
    ma_d = nc.dram_tensor("mA", [128, TT], bf16, kind="ExternalInput").ap()
    mb2_d = nc.dram_tensor("mB2", [128, 2 * TT], bf16,
                           kind="ExternalInput").ap()
    out_d = nc.dram_tensor("out", [TOK, DIN], f32, kind="ExternalOutput").ap()

    with tile.TileContext(nc) as tc:
        with (
            tc.tile_pool(name="persist", bufs=1) as pp,
            tc.tile_pool(name="scratch", bufs=2) as sp,
        ):
            # ---- DMA in (emission order ~ priority) ----
            wq = [pp.tile([128, DIN], io_dt(proj_dt), name=f"wq{k}",
                          tag=f"wq{k}") for k in range(4)]
            xt = [pp.tile([128, TOK], io_dt(proj_dt), name=f"xt{k}",
                          tag=f"xt{k}") for k in range(4)]
            for k in range(4):
                nc.sync.dma_start(out=wq[k], in_=w_d["wq"][k * 128:(k + 1) * 128, :])
                nc.sync.dma_start(out=xt[k], in_=xt_d[k * 128:(k + 1) * 128, :])
            bqc = pp.tile([128, 4], f32, name="bqc", tag="bqc")
            bkc = pp.tile([128, 4], f32, name="bkc", tag="bkc")
            nc.sync.dma_start(out=bqc, in_=bqc_d)
            nc.sync.dma_start(out=bkc, in_=bkc_d)
            wk = [pp.tile([128, DIN], io_dt(proj_dt), name=f"wk{k}",
                          tag=f"wk{k}") for k in range(4)]
            for k in range(4):
                nc.sync.dma_start(out=wk[k], in_=w_d["wk"][k * 128:(k + 1) * 128, :])
            wv = [pp.tile([128, DIN], io_dt(proj_dt), name=f"wv{k}",
                          tag=f"wv{k}") for k in range(4)]
            for k in range(4):
                nc.sync.dma_start(out=wv[k], in_=w_d["wv"][k * 128:(k + 1) * 128, :])
            bvb = pp.tile([128, DIN], f32, name="bvb", tag="bvb")
            nc.sync.dma_start(out=bvb, in_=bvb_d)
            ma = pp.tile([128, TT], bf16, name="ma", tag="ma")
            mb2 = pp.tile([128, 2 * TT], bf16, name="mb2", tag="mb2")
            nc.sync.dma_start(out=ma, in_=ma_d)
            nc.sync.dma_start(out=mb2, in_=mb2_d)

            # ---- persistent activations ----
            qt = [pp.tile([128, TOK], io_dt(qk_dt), name=f"qt{k}",
                          tag=f"qt{k}") for k in range(4)]
            kt_ = [pp.tile([128, TOK], io_dt(qk_dt), name=f"kt{k}",
                           tag=f"kt{k}") for k in range(4)]
            # v with 65 columns per head: col h*65+64 is all-ones so the PV
            # matmul also produces the softmax denominator in its last column
            vt = [pp.tile([TT, H * (D + 1)], io_dt(pv_dt), name=f"vt{t}",
                          tag=f"vt{t}") for t in range(NT)]
            ot = [pp.tile([TT, DIN], f32, name=f"ot{t}", tag=f"ot{t}")
                  for t in range(NT)]

            with (
                tc.tile_pool(name="ppsum", bufs=2, space="PSUM") as pps,
                tc.tile_pool(name="vpsum", bufs=2, space="PSUM") as vps,
            ):
                # ---- q^T / k^T projections: psum[feat, tok] ----
                for (w, bc, dst) in ((wq, bqc, qt), (wk, bkc, kt_)):
                    for ft in range(4):
                        fsl = slice(ft * 128, (ft + 1) * 128)
                        for ch in range(2):
                            csl = slice(ch * 400, (ch + 1) * 400)
                            acc = pps.tile([128, 400], f32, name="pacc",
                                           tag="p", bufs=2)
                            for k in range(4):
                                nc.tensor.matmul(
                                    acc[:], mm(w[k][:, fsl], proj_dt),
                                    mm(xt[k][:, csl], proj_dt),
                                    start=(k == 0), stop=(k == 3))
                            nc.scalar.activation(dst[ft][:, csl], acc[:],
                                                 AF.Identity,
                                                 bias=bc[:, ft:ft + 1])

                # ---- v projection: psum[tok, feat]; bias+relu on DVE ----
                for t in range(NT):
                    tsl = slice(t * TT, (t + 1) * TT)
                    acc = vps.tile([TT, DIN], f32, name="vacc", tag="v",
                                   bufs=2)
                    for k in range(4):
                        nc.tensor.matmul(acc[:], mm(xt[k][:, tsl], proj_dt),
                                         mm(wv[k][:], proj_dt),
                                         start=(k == 0), stop=(k == 3))
                    vdat = vt[t].rearrange("p (h c) -> p h c", c=D + 1)[:, :, :D]
                    vones = vt[t].rearrange("p (h c) -> p h c",
                                            c=D + 1)[:, :, D:D + 1]
                    nc.vector.scalar_tensor_tensor(
                        vdat, acc.rearrange("p (h c) -> p h c", c=D), 0.0,
                        bvb[:TT, :].rearrange("p (h c) -> p h c", c=D),
                        op0=ALU.add, op1=ALU.add)
                    nc.vector.tensor_scalar_max(vdat, vdat, 0.0)
                    nc.vector.memset(vones, 1.0)

                if stage == "proj":
                    for t in range(NT):
                        nc.vector.tensor_copy(ot[t][:], vt[t][:])
                        nc.sync.dma_start(out=out_d[t * TT:(t + 1) * TT, :],
                                          in_=ot[t][:])

            # ---- attention ----
            # Only S^T = K_h^T-stationary @ Q_h is computed (per head, per
            # tile).  E^T = exp(S^T) ⊙ block-mask is the PV stationary; the
            # ones-column of v turns PV's last column into the softmax
            # denominator.  Row-group safety: each PSUM bank only receives
            # matmuls from ONE base-partition (0 or 64); even heads (bp0)
            # and odd heads (bp64) use separate banks so the PE's
            # row-group-concurrent matmuls never co-write a bank.
            if stage != "proj":
                ctx_aps = tc.tile_pool(name="apsum", bufs=6, space="PSUM")
                aps = ctx_aps.__enter__()

            for t in range(NT) if stage != "proj" else []:
                tsl = slice(t * TT, (t + 1) * TT)
                for hg in range(2):
                    heads = [hg * 4, hg * 4 + 1, hg * 4 + 2, hg * 4 + 3]
                    stE = aps.tile([TT, 2 * TT], f32, name="stE", tag="s",
                                   bufs=6)
                    stO = aps.tile([TT, 2 * TT], f32, name="stO", tag="s",
                                   bufs=6)
                    nc.tensor.matmul(stE[:], ma[0:5, :], mb2[0:5, :],
                                     start=True, stop=False,
                                     skip_group_check=True)
                    nc.tensor.matmul(stO[:], ma[64:69, :], mb2[64:69, :],
                                     start=True, stop=False,
                                     skip_group_check=True)
                    for i, h in enumerate(heads):
                        ft, po = h // 2, (h % 2) * 64
                        dst = (stE, stO)[h % 2]
                        col = (i // 2) * TT
                        nc.tensor.matmul(
                            dst[:, col:col + TT],
                            mm(kt_[ft][po:po + 64, tsl], qk_dt),
                            mm(qt[ft][po:po + 64, tsl], qk_dt),
                            start=False, stop=(i >= 2),
                            skip_group_check=True)
                    etE = sp.tile([TT, 2 * TT], io_dt(pv_dt), name="etE",
                                  tag="et", bufs=6)
                    etO = sp.tile([TT, 2 * TT], io_dt(pv_dt), name="etO",
                                  tag="et", bufs=6)
                    nc.scalar.activation(etE[:], stE[:], AF.Exp)
                    nc.scalar.activation(etO[:], stO[:], AF.Exp)

                    # all 4 PVs of this unit share one PSUM bank (all their
                    # matmuls use rows 0-99 -> serialized in order, safe)
                    pv4 = aps.tile([TT, 4 * (D + 1)], f32, name="pv4",
                                   tag="pv", bufs=2)
                    for i, h in enumerate(heads):
                        et = (etE, etO)[h % 2]
                        col = (i // 2) * TT
                        nc.tensor.matmul(pv4[:, i * (D + 1):
                                             (i + 1) * (D + 1)],
                                         mm(et[:, col:col + TT], pv_dt),
                                         mm(vt[t][:, h * (D + 1):
                                                  (h + 1) * (D + 1)], pv_dt),
                                         start=True, stop=True,
                                         skip_group_check=True)
                    pv4v = pv4.rearrange("p (h c) -> p h c", c=D + 1)
                    rc4 = sp.tile([TT, 4], f32, name="rc4", tag="rc", bufs=8)
                    nc.vector.reciprocal(rc4[:], pv4v[:, :, D:D + 1])
                    for i, h in enumerate(heads):
                        nc.vector.tensor_scalar_mul(
                            ot[t][:, h * D:(h + 1) * D],
                            pv4v[:, i, :D], rc4[:, i:i + 1])
                nc.sync.dma_start(out=out_d[tsl, :], in_=ot[t][:])

            if stage != "proj":
                ctx_aps.__exit__(None, None, None)

    nc.compile()
    return nc


def _prep_inputs(x, Wq, bq, Wk, bk, Wv, bv, proj_dt):
    import ml_dtypes

    x = np.asarray(x, np.float32)
    Wq = np.asarray(Wq, np.float32)
    bq = np.asarray(bq, np.float32)
    Wk = np.asarray(Wk, np.float32)
    bk = np.asarray(bk, np.float32)
    Wv = np.asarray(Wv, np.float32)
    bv = np.asarray(bv, np.float32)

    scale = 1.0 / np.sqrt(np.float32(D))  # 1/8, exact
    wq_s = (Wq * scale).astype(np.float32)
    bq_s = (bq * scale).astype(np.float32)

    io_np = ml_dtypes.bfloat16 if proj_dt == "bf16" else np.float32
    xT = np.ascontiguousarray(x.transpose(0, 2, 1))  # [B, DIN, N]

    bqc = np.ascontiguousarray(bq_s.reshape(4, 128).T)
    bkc = np.ascontiguousarray(bk.reshape(4, 128).T)
    bvb = np.ascontiguousarray(np.tile(bv[None, :], (128, 1)))

    # rank-5 factors of the additive frame mask over one 100-token tile,
    # replicated at partition rows 0-4 (bp0 banks) and 64-68 (bp64 banks)
    mA = np.zeros((128, TT), ml_dtypes.bfloat16)
    mB = np.zeros((128, TT), ml_dtypes.bfloat16)
    big = ml_dtypes.bfloat16(9e15)
    for base in (0, 64):
        mA[base, :] = 1
        mB[base, :] = -big
        for f in range(4):
            mA[base + 1 + f, f * JN:(f + 1) * JN] = 1
            mB[base + 1 + f, f * JN:(f + 1) * JN] = big
    mB2 = np.ascontiguousarray(np.tile(mB, (1, 2)))

    in_maps = []
    for c in range(NCORES):
        b, fb = c // 4, c % 4
        in_maps.append({
            "xT": np.ascontiguousarray(
                xT[b, :, fb * TOK:(fb + 1) * TOK]).astype(io_np),
            "wq": wq_s.astype(io_np),
            "wk": Wk.astype(io_np),
            "wv": Wv.astype(io_np),
            "bqc": bqc, "bkc": bkc, "bvb": bvb,
            "mA": mA, "mB2": mB2,
        })
    return in_maps


def kernel(x, Wq, bq, Wk, bk, Wv, bv, att_heads=H, latent_dim=D,
           time_len=TL, joint_num=JN, **_):
    from concourse.bass_utils import run_bass_kernel_spmd

    cfg = tuple(sorted(CONFIG.items()))
    if cfg not in _CACHE:
        _CACHE[cfg] = _build(CONFIG)
    nc = _CACHE[cfg]

    in_maps = _prep_inputs(x, Wq, bq, Wk, bk, Wv, bv, CONFIG["proj"])
    res = run_bass_kernel_spmd(nc, in_maps, core_ids=list(range(NCORES)))
    global LAST_RESULT
    LAST_RESULT = res

    out = np.empty((B, N, DIN), np.float32)
    for c in range(NCORES):
        b, fb = c // 4, c % 4
        out[b, fb * TOK:(fb + 1) * TOK, :] = res.results[c]["out"]
    return out



# revision 3
# speedup vs baseline: 1.0441x; 1.0441x over previous
"""Block-diagonal (per-frame) multi-head attention on 8 Trainium2 cores.

Problem: x[2,3200,512] -> QKV proj (H=8 heads, D=64) -> attention masked to
25-token frames (128 frames) -> out[2,3200,512].  N = 3200 = 128*25.

Sharding: 256 (batch, frame) groups; core c handles batch c//4, frames
(c%4)*32..+32  => 800 tokens/core, tiled as 8 x 100 tokens (4 frames).

Layout trick: host sends x pre-transposed (xT [512, 800]) so every matmul
contracts over the partition dim:
  qT/kT [feat, tok] = W.T @ xT   (lhsT = W slice, rhs = xT)
  v     [tok, feat] = xT.T @ Wv  (lhsT = xT slice, rhs = Wv)
Scores per (head, tile): S = qT_h.T @ kT_h and S^T = kT_h.T @ qT_h -- both
directly available, no transposes anywhere.  The -9e15 frame mask is rank-5
(ones + 4 frame indicators), injected by one small matmul that initializes
the PSUM accumulation group.  softmax skips max-subtraction (|scores| <~ 8).
PV uses E^T = exp(S^T) as the stationary operand with v natural as moving.
"""

import numpy as np

B, N, DIN = 2, 3200, 512
H, D = 8, 64
TL, JN = 128, 25
NCORES = 8
TOK = 800      # tokens per core
NT = 8         # token tiles per core
TT = 100       # tokens per tile (4 frames)
NEG = -9e15

# matmul dtype per stage: 'f32' | 'f32r' | 'bf16'
#   proj: QKV projection matmuls (and dtype of xT/W in SBUF+HBM)
#   qk:   dtype of qT/kT tiles (scores matmuls)
#   pv:   dtype of E^T and V tiles (PV matmul)
CONFIG = {"proj": "f32", "qk": "f32", "pv": "f32"}

_CACHE = {}
LAST_RESULT = None  # BassKernelResults of the most recent kernel() call


def _build(cfg, stage="full"):
    import concourse.bacc as bacc
    import concourse.tile as tile
    from concourse import mybir

    f32 = mybir.dt.float32
    bf16 = mybir.dt.bfloat16
    f16 = mybir.dt.float16
    f32r = mybir.dt.float32r
    AF = mybir.ActivationFunctionType
    ALU = mybir.AluOpType
    AX = mybir.AxisListType

    def io_dt(kind):
        return {"f32": f32, "f32r": f32r, "bf16": bf16, "f16": f16}[kind]

    def mm(ap, kind):
        return ap

    proj_dt, qk_dt, pv_dt = cfg["proj"], cfg["qk"], cfg["pv"]

    nc = bacc.Bacc("TRN2", target_bir_lowering=False, debug=False,
                   num_devices=NCORES)

    xt_d = nc.dram_tensor("xT", [DIN, TOK], io_dt(proj_dt),
                          kind="ExternalInput").ap()
    w_d = {}
    for nm in ("wq", "wk", "wv"):
        w_d[nm] = nc.dram_tensor(nm, [DIN, DIN], io_dt(proj_dt),
                                 kind="ExternalInput").ap()
    bqc_d = nc.dram_tensor("bqc", [128, 4], f32, kind="ExternalInput").ap()
    bkc_d = nc.dram_tensor("bkc", [128, 4], f32, kind="ExternalInput").ap()
    bvb_d = nc.dram_tensor("bvb", [128, DIN], f32, kind="ExternalInput").ap()
    ma_d = nc.dram_tensor("mA", [128, TT], bf16, kind="ExternalInput").ap()
    mb2_d = nc.dram_tensor("mB2", [128, 2 * TT], bf16,
                           kind="ExternalInput").ap()
    out_d = nc.dram_tensor("out", [TOK, DIN], f32, kind="ExternalOutput").ap()

    with tile.TileContext(nc) as tc:
        with (
            tc.tile_pool(name="persist", bufs=1) as pp,
            tc.tile_pool(name="scratch", bufs=2) as sp,
        ):
            # ---- DMA in (emission order ~ priority) ----
            wq = [pp.tile([128, DIN], io_dt(proj_dt), name=f"wq{k}",
                          tag=f"wq{k}") for k in range(4)]
            xt = [pp.tile([128, TOK], io_dt(proj_dt), name=f"xt{k}",
                          tag=f"xt{k}") for k in range(4)]
            for k in range(4):
                nc.sync.dma_start(out=wq[k], in_=w_d["wq"][k * 128:(k + 1) * 128, :])
                nc.sync.dma_start(out=xt[k], in_=xt_d[k * 128:(k + 1) * 128, :])
            bqc = pp.tile([128, 4], f32, name="bqc", tag="bqc")
            bkc = pp.tile([128, 4], f32, name="bkc", tag="bkc")
            nc.sync.dma_start(out=bqc, in_=bqc_d)
            nc.sync.dma_start(out=bkc, in_=bkc_d)
            wk = [pp.tile([128, DIN], io_dt(proj_dt), name=f"wk{k}",
                          tag=f"wk{k}") for k in range(4)]
            for k in range(4):
                nc.sync.dma_start(out=wk[k], in_=w_d["wk"][k * 128:(k + 1) * 128, :])
            wv = [pp.tile([128, DIN], io_dt(proj_dt), name=f"wv{k}",
                          tag=f"wv{k}") for k in range(4)]
            for k in range(4):
                nc.sync.dma_start(out=wv[k], in_=w_d["wv"][k * 128:(k + 1) * 128, :])
            bvb = pp.tile([128, DIN], f32, name="bvb", tag="bvb")
            nc.sync.dma_start(out=bvb, in_=bvb_d)
            ma = pp.tile([128, TT], bf16, name="ma", tag="ma")
            mb2 = pp.tile([128, 2 * TT], bf16, name="mb2", tag="mb2")
            nc.sync.dma_start(out=ma, in_=ma_d)
            nc.sync.dma_start(out=mb2, in_=mb2_d)

            # ---- persistent activations ----
            qt = [pp.tile([128, TOK], io_dt(qk_dt), name=f"qt{k}",
                          tag=f"qt{k}") for k in range(4)]
            kt_ = [pp.tile([128, TOK], io_dt(qk_dt), name=f"kt{k}",
                           tag=f"kt{k}") for k in range(4)]
            # v with 65 columns per head: col h*65+64 is all-ones so the PV
            # matmul also produces the softmax denominator in its last column
            vt = [pp.tile([TT, H * (D + 1)], io_dt(pv_dt), name=f"vt{t}",
                          tag=f"vt{t}") for t in range(NT)]
            ot = [pp.tile([TT, DIN], f32, name=f"ot{t}", tag=f"ot{t}")
                  for t in range(NT)]

            with (
                tc.tile_pool(name="ppsum", bufs=2, space="PSUM") as pps,
                tc.tile_pool(name="vpsum", bufs=2, space="PSUM") as vps,
            ):
                # ---- q^T / k^T projections: psum[feat, tok] ----
                for (w, bc, dst) in ((wq, bqc, qt), (wk, bkc, kt_)):
                    for ft in range(4):
                        fsl = slice(ft * 128, (ft + 1) * 128)
                        for ch in range(2):
                            csl = slice(ch * 400, (ch + 1) * 400)
                            acc = pps.tile([128, 400], f32, name="pacc",
                                           tag="p", bufs=2)
                            for k in range(4):
                                nc.tensor.matmul(
                                    acc[:], mm(w[k][:, fsl], proj_dt),
                                    mm(xt[k][:, csl], proj_dt),
                                    start=(k == 0), stop=(k == 3))
                            nc.scalar.activation(dst[ft][:, csl], acc[:],
                                                 AF.Identity,
                                                 bias=bc[:, ft:ft + 1])

                # ---- v projection: psum[tok, feat]; bias+relu on DVE ----
                for t in range(NT):
                    tsl = slice(t * TT, (t + 1) * TT)
                    acc = vps.tile([TT, DIN], f32, name="vacc", tag="v",
                                   bufs=2)
                    for k in range(4):
                        nc.tensor.matmul(acc[:], mm(xt[k][:, tsl], proj_dt),
                                         mm(wv[k][:], proj_dt),
                                         start=(k == 0), stop=(k == 3))
                    vdat = vt[t].rearrange("p (h c) -> p h c", c=D + 1)[:, :, :D]
                    vones = vt[t].rearrange("p (h c) -> p h c",
                                            c=D + 1)[:, :, D:D + 1]
                    nc.vector.scalar_tensor_tensor(
                        vdat, acc.rearrange("p (h c) -> p h c", c=D), 0.0,
                        bvb[:TT, :].rearrange("p (h c) -> p h c", c=D),
                        op0=ALU.add, op1=ALU.add)
                    nc.vector.tensor_scalar_max(vdat, vdat, 0.0)
                    nc.vector.memset(vones, 1.0)

                if stage == "proj":
                    for t in range(NT):
                        nc.vector.tensor_copy(ot[t][:], vt[t][:])
                        nc.sync.dma_start(out=out_d[t * TT:(t + 1) * TT, :],
                                          in_=ot[t][:])

            # ---- attention ----
            # Only S^T = K_h^T-stationary @ Q_h is computed (per head, per
            # tile).  E^T = exp(S^T) ⊙ block-mask is the PV stationary; the
            # ones-column of v turns PV's last column into the softmax
            # denominator.  Row-group safety: each PSUM bank only receives
            # matmuls from ONE base-partition (0 or 64); even heads (bp0)
            # and odd heads (bp64) use separate banks so the PE's
            # row-group-concurrent matmuls never co-write a bank.
            if stage != "proj":
                ctx_aps = tc.tile_pool(name="apsum", bufs=6, space="PSUM")
                aps = ctx_aps.__enter__()

            for t in range(NT) if stage != "proj" else []:
                tsl = slice(t * TT, (t + 1) * TT)
                for hg in range(2):
                    heads = [hg * 4, hg * 4 + 1, hg * 4 + 2, hg * 4 + 3]
                    stE = aps.tile([TT, 2 * TT], f32, name="stE", tag="s",
                                   bufs=6)
                    stO = aps.tile([TT, 2 * TT], f32, name="stO", tag="s",
                                   bufs=6)
                    nc.tensor.matmul(stE[:], ma[0:5, :], mb2[0:5, :],
                                     start=True, stop=False,
                                     skip_group_check=True)
                    nc.tensor.matmul(stO[:], ma[64:69, :], mb2[64:69, :],
                                     start=True, stop=False,
                                     skip_group_check=True)
                    for i, h in enumerate(heads):
                        ft, po = h // 2, (h % 2) * 64
                        dst = (stE, stO)[h % 2]
                        col = (i // 2) * TT
                        nc.tensor.matmul(
                            dst[:, col:col + TT],
                            mm(kt_[ft][po:po + 64, tsl], qk_dt),
                            mm(qt[ft][po:po + 64, tsl], qk_dt),
                            start=False, stop=(i >= 2),
                            skip_group_check=True)
                    etE = sp.tile([TT, 2 * TT], io_dt(pv_dt), name="etE",
                                  tag="et", bufs=6)
                    etO = sp.tile([TT, 2 * TT], io_dt(pv_dt), name="etO",
                                  tag="et", bufs=6)
                    nc.scalar.activation(etE[:], stE[:], AF.Exp)
                    nc.scalar.activation(etO[:], stO[:], AF.Exp)

                    # all 4 PVs of this unit share one PSUM bank (all their
                    # matmuls use rows 0-99 -> serialized in order, safe)
                    pv4 = aps.tile([TT, 4 * (D + 1)], f32, name="pv4",
                                   tag="pv", bufs=2)
                    for i, h in enumerate(heads):
                        et = (etE, etO)[h % 2]
                        col = (i // 2) * TT
                        nc.tensor.matmul(pv4[:, i * (D + 1):
                                             (i + 1) * (D + 1)],
                                         mm(et[:, col:col + TT], pv_dt),
                                         mm(vt[t][:, h * (D + 1):
                                                  (h + 1) * (D + 1)], pv_dt),
                                         start=True, stop=True,
                                         skip_group_check=True)
                    pv4v = pv4.rearrange("p (h c) -> p h c", c=D + 1)
                    rc4 = sp.tile([TT, 4], f32, name="rc4", tag="rc", bufs=8)
                    nc.vector.reciprocal(rc4[:], pv4v[:, :, D:D + 1])
                    for i, h in enumerate(heads):
                        nc.vector.tensor_scalar_mul(
                            ot[t][:, h * D:(h + 1) * D],
                            pv4v[:, i, :D], rc4[:, i:i + 1])
                nc.sync.dma_start(out=out_d[tsl, :], in_=ot[t][:])

            if stage != "proj":
                ctx_aps.__exit__(None, None, None)

    nc.compile()
    return nc


def _prep_inputs(x, Wq, bq, Wk, bk, Wv, bv, proj_dt):
    import ml_dtypes

    x = np.asarray(x, np.float32)
    Wq = np.asarray(Wq, np.float32)
    bq = np.asarray(bq, np.float32)
    Wk = np.asarray(Wk, np.float32)
    bk = np.asarray(bk, np.float32)
    Wv = np.asarray(Wv, np.float32)
    bv = np.asarray(bv, np.float32)

    scale = 1.0 / np.sqrt(np.float32(D))  # 1/8, exact
    wq_s = (Wq * scale).astype(np.float32)
    bq_s = (bq * scale).astype(np.float32)

    io_np = {"bf16": ml_dtypes.bfloat16, "f16": np.float16}.get(proj_dt, np.float32)
    xT = np.ascontiguousarray(x.transpose(0, 2, 1))  # [B, DIN, N]

    bqc = np.ascontiguousarray(bq_s.reshape(4, 128).T)
    bkc = np.ascontiguousarray(bk.reshape(4, 128).T)
    bvb = np.ascontiguousarray(np.tile(bv[None, :], (128, 1)))

    # rank-5 factors of the additive frame mask over one 100-token tile,
    # replicated at partition rows 0-4 (bp0 banks) and 64-68 (bp64 banks)
    mA = np.zeros((128, TT), ml_dtypes.bfloat16)
    mB = np.zeros((128, TT), ml_dtypes.bfloat16)
    big = ml_dtypes.bfloat16(9e15)
    for base in (0, 64):
        mA[base, :] = 1
        mB[base, :] = -big
        for f in range(4):
            mA[base + 1 + f, f * JN:(f + 1) * JN] = 1
            mB[base + 1 + f, f * JN:(f + 1) * JN] = big
    mB2 = np.ascontiguousarray(np.tile(mB, (1, 2)))

    in_maps = []
    for c in range(NCORES):
        b, fb = c // 4, c % 4
        in_maps.append({
            "xT": np.ascontiguousarray(
                xT[b, :, fb * TOK:(fb + 1) * TOK]).astype(io_np),
            "wq": wq_s.astype(io_np),
            "wk": Wk.astype(io_np),
            "wv": Wv.astype(io_np),
            "bqc": bqc, "bkc": bkc, "bvb": bvb,
            "mA": mA, "mB2": mB2,
        })
    return in_maps


def kernel(x, Wq, bq, Wk, bk, Wv, bv, att_heads=H, latent_dim=D,
           time_len=TL, joint_num=JN, **_):
    from concourse.bass_utils import run_bass_kernel_spmd

    cfg = tuple(sorted(CONFIG.items()))
    if cfg not in _CACHE:
        _CACHE[cfg] = _build(CONFIG)
    nc = _CACHE[cfg]

    in_maps = _prep_inputs(x, Wq, bq, Wk, bk, Wv, bv, CONFIG["proj"])
    res = run_bass_kernel_spmd(nc, in_maps, core_ids=list(range(NCORES)))
    global LAST_RESULT
    LAST_RESULT = res

    out = np.empty((B, N, DIN), np.float32)
    for c in range(NCORES):
        b, fb = c // 4, c % 4
        out[b, fb * TOK:(fb + 1) * TOK, :] = res.results[c]["out"]
    return out



# revision 4
# speedup vs baseline: 1.2154x; 1.1641x over previous
"""Block-diagonal (per-frame) multi-head attention on 8 Trainium2 cores.

Problem: x[2,3200,512] -> QKV proj (H=8 heads, D=64) -> attention masked to
25-token frames (128 frames) -> out[2,3200,512].  N = 3200 = 128*25.

Sharding: 256 (batch, frame) groups; core c handles batch c//4, frames
(c%4)*32..+32  => 800 tokens/core, tiled as 8 x 100 tokens (4 frames).

v2 layout/schedule:
  - All stages in ONE pool scope so the Tile scheduler can overlap the
    QKV projections with attention tiles (no phase barrier).
  - Projections contract over the partition dim: qT/kT [feat, tok] =
    W.T @ xT with W-slices stationary; v [tok, feat] = xT.T @ Wv.
  - Per 100-token tile, scores live in TWO psum banks: stE [100, 4*100]
    holds the 4 even heads (PE rows 0-63), stO the odd heads (rows
    64-127) -- separate banks so the PE's row-group-concurrent matmuls
    never co-write a bank.  A rank-5 mask matmul (f16-safe +-30000)
    initializes each bank; exp is ONE activation per bank.
  - v has a ones-column per head so PV's last column yields the softmax
    denominator; per tile ONE reciprocal + ONE broadcast multiply
    produce the normalized output.
  - Outputs DMA on the gpsimd queue so they don't head-block inputs.
"""

import numpy as np

B, N, DIN = 2, 3200, 512
H, D = 8, 64
TL, JN = 128, 25
NCORES = 8
TOK = 800      # tokens per core
NT = 8         # token tiles per core
TT = 100       # tokens per tile (4 frames)
CH = 400       # proj column-chunk (2 chunks)
NEGB = 30000.0  # additive mask magnitude (f16-safe; |scores| <~ 10)

# matmul dtype per stage: 'f32' | 'f32r' | 'bf16' | 'f16'
CONFIG = {"proj": "f32r", "qk": "f16", "pv": "f16"}

_CACHE = {}
LAST_RESULT = None  # BassKernelResults of the most recent kernel() call


def _build(cfg):
    import concourse.bacc as bacc
    import concourse.tile as tile
    from concourse import mybir
    from concourse.bass import broadcast_tensor_aps

    f32 = mybir.dt.float32
    bf16 = mybir.dt.bfloat16
    f16 = mybir.dt.float16
    f32r = mybir.dt.float32r
    AF = mybir.ActivationFunctionType
    ALU = mybir.AluOpType

    def io_dt(kind):
        return {"f32": f32, "f32r": f32r, "bf16": bf16, "f16": f16}[kind]

    proj_dt = io_dt(cfg["proj"])
    qk_dt = io_dt(cfg["qk"])
    pv_dt = io_dt(cfg["pv"])
    mask_dt = f16 if cfg["qk"] == "f16" else bf16

    nc = bacc.Bacc("TRN2", target_bir_lowering=False, debug=False,
                   num_devices=NCORES)

    xt_d = nc.dram_tensor("xT", [DIN, TOK], proj_dt,
                          kind="ExternalInput").ap()
    w_d = {}
    for nm in ("wq", "wk", "wv"):
        w_d[nm] = nc.dram_tensor(nm, [DIN, DIN], proj_dt,
                                 kind="ExternalInput").ap()
    bqc_d = nc.dram_tensor("bqc", [128, 4], f32, kind="ExternalInput").ap()
    bkc_d = nc.dram_tensor("bkc", [128, 4], f32, kind="ExternalInput").ap()
    bvb_d = nc.dram_tensor("bvb", [128, DIN], f32, kind="ExternalInput").ap()
    ma_d = nc.dram_tensor("mA", [128, TT], mask_dt, kind="ExternalInput").ap()
    mb4_d = nc.dram_tensor("mB4", [128, 4 * TT], mask_dt,
                           kind="ExternalInput").ap()
    out_d = nc.dram_tensor("out", [TOK, DIN], f32, kind="ExternalOutput").ap()

    with tile.TileContext(nc) as tc:
        with (
            tc.tile_pool(name="pp", bufs=1) as pp,
            tc.tile_pool(name="sp", bufs=4) as sp,
            tc.tile_pool(name="ps", bufs=2, space="PSUM") as ps,
        ):
            # ---- persistent tiles ----
            wq = [pp.tile([128, DIN], proj_dt, name=f"wq{k}", tag=f"wq{k}")
                  for k in range(4)]
            wk = [pp.tile([128, DIN], proj_dt, name=f"wk{k}", tag=f"wk{k}")
                  for k in range(4)]
            wv = [pp.tile([128, DIN], proj_dt, name=f"wv{k}", tag=f"wv{k}")
                  for k in range(4)]
            xt = [pp.tile([128, TOK], proj_dt, name=f"xt{k}", tag=f"xt{k}")
                  for k in range(4)]
            bqc = pp.tile([128, 4], f32, name="bqc", tag="bqc")
            bkc = pp.tile([128, 4], f32, name="bkc", tag="bkc")
            bvb = pp.tile([128, DIN], f32, name="bvb", tag="bvb")
            ma = pp.tile([128, TT], mask_dt, name="ma", tag="ma")
            mb4 = pp.tile([128, 4 * TT], mask_dt, name="mb4", tag="mb4")

            qt = [pp.tile([128, TOK], qk_dt, name=f"qt{k}", tag=f"qt{k}")
                  for k in range(4)]
            kt_ = [pp.tile([128, TOK], qk_dt, name=f"kt{k}", tag=f"kt{k}")
                   for k in range(4)]
            # v with 65 columns per head: col h*65+64 is all-ones so the PV
            # matmul also produces the softmax denominator in its last column
            vt = [pp.tile([TT, H * (D + 1)], pv_dt, name=f"vt{t}",
                          tag=f"vt{t}") for t in range(NT)]
            ot = [pp.tile([TT, DIN], f32, name=f"ot{t}", tag=f"ot{t}")
                  for t in range(NT)]

            # ---- input DMAs in priority order (one hw queue, in-order) ----
            for k in range(4):
                nc.sync.dma_start(out=wq[k],
                                  in_=w_d["wq"][k * 128:(k + 1) * 128, :])
                nc.sync.dma_start(out=xt[k][:, 0:CH],
                                  in_=xt_d[k * 128:(k + 1) * 128, 0:CH])
            nc.sync.dma_start(out=bqc, in_=bqc_d)
            nc.sync.dma_start(out=bkc, in_=bkc_d)
            for k in range(4):
                nc.sync.dma_start(out=wk[k],
                                  in_=w_d["wk"][k * 128:(k + 1) * 128, :])
            nc.sync.dma_start(out=ma, in_=ma_d)
            nc.sync.dma_start(out=mb4, in_=mb4_d)
            nc.sync.dma_start(out=bvb, in_=bvb_d)
            for k in range(4):
                nc.sync.dma_start(out=wv[k],
                                  in_=w_d["wv"][k * 128:(k + 1) * 128, :])
            for k in range(4):
                nc.sync.dma_start(out=xt[k][:, CH:TOK],
                                  in_=xt_d[k * 128:(k + 1) * 128, CH:TOK])

            # ---- stage emitters ----
            def qk_group(w, bc, dst, ft, ch):
                fsl = slice(ft * 128, (ft + 1) * 128)
                csl = slice(ch * CH, (ch + 1) * CH)
                acc = ps.tile([128, CH], f32, name="acc", tag="acc", bufs=2)
                for k in range(4):
                    nc.tensor.matmul(acc[:], w[k][:, fsl], xt[k][:, csl],
                                     start=(k == 0), stop=(k == 3))
                nc.scalar.activation(dst[ft][:, csl], acc[:], AF.Identity,
                                     bias=bc[:, ft:ft + 1])

            def v_tile(t):
                tsl = slice(t * TT, (t + 1) * TT)
                acc = ps.tile([TT, DIN], f32, name="vacc", tag="vacc", bufs=2)
                for k in range(4):
                    nc.tensor.matmul(acc[:], xt[k][:, tsl], wv[k][:],
                                     start=(k == 0), stop=(k == 3))
                vv = vt[t].rearrange("p (h c) -> p h c", c=D + 1)
                av = acc.rearrange("p (h c) -> p h c", c=D)
                bv = bvb[:TT, :].rearrange("p (h c) -> p h c", c=D)
                nc.vector.scalar_tensor_tensor(vv[:, :, :D], av, 0.0, bv,
                                               op0=ALU.add, op1=ALU.add)
                nc.vector.tensor_scalar_max(vv[:, :, :D], vv[:, :, :D], 0.0)
                nc.vector.memset(vv[:, :, D:D + 1], 1.0)

            def att_tile(t):
                tsl = slice(t * TT, (t + 1) * TT)
                # two banks: even heads (PE rows 0-63) / odd heads (64-127)
                stE = ps.tile([TT, 4 * TT], f32, name="stE", tag="st",
                              bufs=2)
                stO = ps.tile([TT, 4 * TT], f32, name="stO", tag="st",
                              bufs=2)
                nc.tensor.matmul(stE[:], ma[0:5, :], mb4[0:5, :],
                                 start=True, stop=False,
                                 skip_group_check=True)
                nc.tensor.matmul(stO[:], ma[64:69, :], mb4[64:69, :],
                                 start=True, stop=False,
                                 skip_group_check=True)
                for i in range(4):
                    c = slice(i * TT, (i + 1) * TT)
                    # head 2i: ft=i rows 0-63; head 2i+1: ft=i rows 64-127
                    nc.tensor.matmul(stE[:, c], kt_[i][0:64, tsl],
                                     qt[i][0:64, tsl],
                                     start=False, stop=(i == 3),
                                     skip_group_check=True)
                    nc.tensor.matmul(stO[:, c], kt_[i][64:128, tsl],
                                     qt[i][64:128, tsl],
                                     start=False, stop=(i == 3),
                                     skip_group_check=True)
                etE = sp.tile([TT, 4 * TT], pv_dt, name="etE", tag="et",
                              bufs=4)
                etO = sp.tile([TT, 4 * TT], pv_dt, name="etO", tag="et",
                              bufs=4)
                nc.scalar.activation(etE[:], stE[:], AF.Exp)
                nc.scalar.activation(etO[:], stO[:], AF.Exp)

                # PV: 2-bank psum [100, 2x512]; head h at bank h//4,
                # col (h%4)*65 (65 cols incl denominator)
                pv = ps.tile([TT, 1024], f32, name="pv", tag="pv", bufs=1)
                for h in range(H):
                    et = etE if h % 2 == 0 else etO
                    blk = h // 2
                    off = (h // 4) * 512 + (h % 4) * 65
                    nc.tensor.matmul(pv[:, off:off + 65],
                                     et[:, blk * TT:(blk + 1) * TT],
                                     vt[t][:, h * 65:(h + 1) * 65],
                                     start=True, stop=True,
                                     skip_group_check=True)
                pvb = pv.rearrange("p (b s) -> p b s", s=512)
                pvq = pvb[:, :, 0:4 * 65].rearrange("p b (q c) -> p b q c",
                                                    c=65)
                rc = sp.tile([TT, 8], f32, name="rc", tag="rc", bufs=4)
                rcv = rc.rearrange("p (b q c) -> p b q c", b=2, c=1)
                nc.vector.reciprocal(rcv, pvq[:, :, :, D:D + 1])
                ov = ot[t].rearrange("p (b q c) -> p b q c", b=2, c=D)
                i0, i1 = broadcast_tensor_aps(pvq[:, :, :, 0:D], rcv)
                nc.vector.tensor_tensor(ov, i0, i1, op=ALU.mult)
                nc.gpsimd.dma_start(out=out_d[tsl, :], in_=ot[t][:])

            # ---- pipelined emission ----
            for ft in range(4):
                qk_group(wq, bqc, qt, ft, 0)
            for ft in range(4):
                qk_group(wk, bkc, kt_, ft, 0)
            v_tile(0)
            v_tile(1)
            v_tile(2)
            att_tile(0)
            v_tile(3)
            att_tile(1)
            qk_group(wq, bqc, qt, 0, 1)
            att_tile(2)
            qk_group(wq, bqc, qt, 1, 1)
            att_tile(3)
            qk_group(wq, bqc, qt, 2, 1)
            qk_group(wq, bqc, qt, 3, 1)
            for ft in range(4):
                qk_group(wk, bkc, kt_, ft, 1)
            v_tile(4)
            v_tile(5)
            v_tile(6)
            att_tile(4)
            v_tile(7)
            att_tile(5)
            att_tile(6)
            att_tile(7)

    nc.compile()
    return nc


def _prep_inputs(x, Wq, bq, Wk, bk, Wv, bv, cfg):
    import ml_dtypes

    x = np.asarray(x, np.float32)
    Wq = np.asarray(Wq, np.float32)
    bq = np.asarray(bq, np.float32)
    Wk = np.asarray(Wk, np.float32)
    bk = np.asarray(bk, np.float32)
    Wv = np.asarray(Wv, np.float32)
    bv = np.asarray(bv, np.float32)

    scale = 1.0 / np.sqrt(np.float32(D))  # 1/8, exact
    wq_s = (Wq * scale).astype(np.float32)
    bq_s = (bq * scale).astype(np.float32)

    io_np = {"bf16": ml_dtypes.bfloat16,
             "f16": np.float16}.get(cfg["proj"], np.float32)
    mask_np = np.float16 if cfg["qk"] == "f16" else ml_dtypes.bfloat16
    xT = np.ascontiguousarray(x.transpose(0, 2, 1))  # [B, DIN, N]

    bqc = np.ascontiguousarray(bq_s.reshape(4, 128).T)
    bkc = np.ascontiguousarray(bk.reshape(4, 128).T)
    bvb = np.ascontiguousarray(np.tile(bv[None, :], (128, 1)))

    # rank-5 factors of the additive frame mask over one 100-token tile,
    # replicated at partition rows 0-4 (even-head bank) and 64-68 (odd)
    big = mask_np(NEGB)
    mA = np.zeros((128, TT), mask_np)
    mB = np.zeros((128, TT), mask_np)
    for base in (0, 64):
        mA[base, :] = 1
        mB[base, :] = -big
        for f in range(4):
            mA[base + 1 + f, f * JN:(f + 1) * JN] = 1
            mB[base + 1 + f, f * JN:(f + 1) * JN] = big
    mB4 = np.ascontiguousarray(np.tile(mB, (1, 4)))

    in_maps = []
    for c in range(NCORES):
        b, fb = c // 4, c % 4
        in_maps.append({
            "xT": np.ascontiguousarray(
                xT[b, :, fb * TOK:(fb + 1) * TOK]).astype(io_np),
            "wq": wq_s.astype(io_np),
            "wk": Wk.astype(io_np),
            "wv": Wv.astype(io_np),
            "bqc": bqc, "bkc": bkc, "bvb": bvb,
            "mA": mA, "mB4": mB4,
        })
    return in_maps


def kernel(x, Wq, bq, Wk, bk, Wv, bv, att_heads=H, latent_dim=D,
           time_len=TL, joint_num=JN, **_):
    from concourse.bass_utils import run_bass_kernel_spmd

    cfg = tuple(sorted(CONFIG.items()))
    if cfg not in _CACHE:
        _CACHE[cfg] = _build(CONFIG)
    nc = _CACHE[cfg]

    in_maps = _prep_inputs(x, Wq, bq, Wk, bk, Wv, bv, CONFIG)
    res = run_bass_kernel_spmd(nc, in_maps, core_ids=list(range(NCORES)))
    global LAST_RESULT
    LAST_RESULT = res

    out = np.empty((B, N, DIN), np.float32)
    for c in range(NCORES):
        b, fb = c // 4, c % 4
        out[b, fb * TOK:(fb + 1) * TOK, :] = res.results[c]["out"]
    return out


# revision 6
# speedup vs baseline: 1.3186x; 1.0849x over previous
"""Block-diagonal (per-frame) multi-head attention on 8 Trainium2 cores.

Problem: x[2,3200,512] -> QKV proj (H=8 heads, D=64) -> attention masked to
25-token frames (128 frames) -> out[2,3200,512].  N = 3200 = 128*25.

Sharding: 256 (batch, frame) groups; core c handles batch c//4, frames
(c%4)*32..+32  => 800 tokens/core, tiled as 8 x 100 tokens (4 frames).

v2 layout/schedule:
  - All stages in ONE pool scope so the Tile scheduler can overlap the
    QKV projections with attention tiles (no phase barrier).
  - Projections contract over the partition dim: qT/kT [feat, tok] =
    W.T @ xT with W-slices stationary; v [tok, feat] = xT.T @ Wv.
  - Per 100-token tile, scores live in TWO psum banks: stE [100, 4*100]
    holds the 4 even heads (PE rows 0-63), stO the odd heads (rows
    64-127) -- separate banks so the PE's row-group-concurrent matmuls
    never co-write a bank.  A rank-5 mask matmul (f16-safe +-30000)
    initializes each bank; exp is ONE activation per bank.
  - v has a ones-column per head so PV's last column yields the softmax
    denominator; per tile ONE reciprocal + ONE broadcast multiply
    produce the normalized output.
  - Outputs DMA on the gpsimd queue so they don't head-block inputs.
"""

import numpy as np

B, N, DIN = 2, 3200, 512
H, D = 8, 64
TL, JN = 128, 25
NCORES = 8
TOK = 800      # tokens per core
NT = 8         # token tiles per core
TT = 100       # tokens per tile (4 frames)
CH = 400       # proj column-chunk (2 chunks)
NEGB = 30000.0  # additive mask magnitude (f16-safe; |scores| <~ 10)

# matmul dtype per stage: 'f32' | 'f32r' | 'bf16' | 'f16'
CONFIG = {"proj": "f16", "qk": "f16", "pv": "f16"}
NWARM = 64     # PE-warmup filler matmuls during the input-DMA lead-in

_CACHE = {}
LAST_RESULT = None  # BassKernelResults of the most recent kernel() call


def _build(cfg):
    import concourse.bacc as bacc
    import concourse.tile as tile
    from concourse import mybir
    from concourse.bass import broadcast_tensor_aps

    f32 = mybir.dt.float32
    bf16 = mybir.dt.bfloat16
    f16 = mybir.dt.float16
    f32r = mybir.dt.float32r
    AF = mybir.ActivationFunctionType
    ALU = mybir.AluOpType

    def io_dt(kind):
        return {"f32": f32, "f32r": f32r, "bf16": bf16, "f16": f16}[kind]

    proj_dt = io_dt(cfg["proj"])
    qk_dt = io_dt(cfg["qk"])
    pv_dt = io_dt(cfg["pv"])
    mask_dt = f16 if cfg["qk"] == "f16" else bf16

    nc = bacc.Bacc("TRN2", target_bir_lowering=False, debug=False,
                   num_devices=NCORES)

    xt_d = nc.dram_tensor("xT", [DIN, TOK], proj_dt,
                          kind="ExternalInput").ap()
    w_d = {}
    for nm in ("wq", "wk", "wv"):
        w_d[nm] = nc.dram_tensor(nm, [DIN, DIN], proj_dt,
                                 kind="ExternalInput").ap()
    bqc_d = nc.dram_tensor("bqc", [128, 4], f32, kind="ExternalInput").ap()
    bkc_d = nc.dram_tensor("bkc", [128, 4], f32, kind="ExternalInput").ap()
    bvb_d = nc.dram_tensor("bvb", [128, DIN], f32, kind="ExternalInput").ap()
    ma_d = nc.dram_tensor("mA", [128, TT], mask_dt, kind="ExternalInput").ap()
    mb4_d = nc.dram_tensor("mB4", [128, 4 * TT], mask_dt,
                           kind="ExternalInput").ap()
    out_d = nc.dram_tensor("out", [TOK, DIN], f32, kind="ExternalOutput").ap()

    with tile.TileContext(nc) as tc:
        with (
            tc.tile_pool(name="pp", bufs=1) as pp,
            tc.tile_pool(name="sp", bufs=4) as sp,
            tc.tile_pool(name="ps", bufs=2, space="PSUM") as ps,
        ):
            # ---- persistent tiles ----
            wq = [pp.tile([128, DIN], proj_dt, name=f"wq{k}", tag=f"wq{k}")
                  for k in range(4)]
            wk = [pp.tile([128, DIN], proj_dt, name=f"wk{k}", tag=f"wk{k}")
                  for k in range(4)]
            wv = [pp.tile([128, DIN], proj_dt, name=f"wv{k}", tag=f"wv{k}")
                  for k in range(4)]
            xt = [pp.tile([128, TOK], proj_dt, name=f"xt{k}", tag=f"xt{k}")
                  for k in range(4)]
            bqc = pp.tile([128, 4], f32, name="bqc", tag="bqc")
            bkc = pp.tile([128, 4], f32, name="bkc", tag="bkc")
            bvb = pp.tile([128, DIN], f32, name="bvb", tag="bvb")
            ma = pp.tile([128, TT], mask_dt, name="ma", tag="ma")
            mb4 = pp.tile([128, 4 * TT], mask_dt, name="mb4", tag="mb4")

            qt = [pp.tile([128, TOK], qk_dt, name=f"qt{k}", tag=f"qt{k}")
                  for k in range(4)]
            kt_ = [pp.tile([128, TOK], qk_dt, name=f"kt{k}", tag=f"kt{k}")
                   for k in range(4)]
            # v with 65 columns per head: col h*65+64 is all-ones so the PV
            # matmul also produces the softmax denominator in its last column
            vt = [pp.tile([TT, H * (D + 1)], pv_dt, name=f"vt{t}",
                          tag=f"vt{t}") for t in range(NT)]
            ot = [pp.tile([TT, DIN], f32, name=f"ot{t}", tag=f"ot{t}")
                  for t in range(NT)]

            # ---- PE warm-up: junk matmuls keep the PE HAM-busy from t~0
            # so the clock is at 8/8 when real work arrives.  They write a
            # psum slot ('pv' tag) whose first real use is ~15us in, and
            # read a memset tile, so they gate nothing.
            junk = pp.tile([128, 256], qk_dt, name="junk", tag="junk")
            nc.vector.memset(junk[:], 0.0)
            wacc = ps.tile([TT, 1024], f32, name="wacc", tag="pv", bufs=1)
            for i in range(NWARM):
                nc.tensor.matmul(wacc[:, 0:256], junk[:, 0:TT],
                                 junk[:, 0:256], start=True, stop=True,
                                 skip_group_check=True)

            # ---- input DMAs in priority order (one hw queue, in-order) ----
            for k in range(4):
                nc.sync.dma_start(out=wq[k],
                                  in_=w_d["wq"][k * 128:(k + 1) * 128, :])
                nc.sync.dma_start(out=xt[k][:, 0:CH],
                                  in_=xt_d[k * 128:(k + 1) * 128, 0:CH])
            nc.sync.dma_start(out=bqc, in_=bqc_d)
            nc.sync.dma_start(out=bkc, in_=bkc_d)
            for k in range(4):
                nc.sync.dma_start(out=wk[k],
                                  in_=w_d["wk"][k * 128:(k + 1) * 128, :])
            nc.sync.dma_start(out=ma, in_=ma_d)
            nc.sync.dma_start(out=mb4, in_=mb4_d)
            nc.sync.dma_start(out=bvb, in_=bvb_d)
            for k in range(4):
                nc.sync.dma_start(out=wv[k],
                                  in_=w_d["wv"][k * 128:(k + 1) * 128, :])
            for k in range(4):
                nc.sync.dma_start(out=xt[k][:, CH:TOK],
                                  in_=xt_d[k * 128:(k + 1) * 128, CH:TOK])

            # ---- stage emitters ----
            def qk_group(w, bc, dst, ft, ch):
                fsl = slice(ft * 128, (ft + 1) * 128)
                csl = slice(ch * CH, (ch + 1) * CH)
                acc = ps.tile([128, CH], f32, name="acc", tag="acc", bufs=2)
                for k in range(4):
                    nc.tensor.matmul(acc[:], w[k][:, fsl], xt[k][:, csl],
                                     start=(k == 0), stop=(k == 3))
                nc.scalar.activation(dst[ft][:, csl], acc[:], AF.Identity,
                                     bias=bc[:, ft:ft + 1])

            def v_tile(t):
                tsl = slice(t * TT, (t + 1) * TT)
                acc = ps.tile([TT, DIN], f32, name="vacc", tag="vacc", bufs=2)
                for k in range(4):
                    nc.tensor.matmul(acc[:], xt[k][:, tsl], wv[k][:],
                                     start=(k == 0), stop=(k == 3))
                vv = vt[t].rearrange("p (h c) -> p h c", c=D + 1)
                av = acc.rearrange("p (h c) -> p h c", c=D)
                bv = bvb[:TT, :].rearrange("p (h c) -> p h c", c=D)
                nc.vector.scalar_tensor_tensor(vv[:, :, :D], av, 0.0, bv,
                                               op0=ALU.add, op1=ALU.add)
                nc.vector.tensor_scalar_max(vv[:, :, :D], vv[:, :, :D], 0.0)
                nc.vector.memset(vv[:, :, D:D + 1], 1.0)

            def att_tile(t):
                tsl = slice(t * TT, (t + 1) * TT)
                # two banks: even heads (PE rows 0-63) / odd heads (64-127)
                stE = ps.tile([TT, 4 * TT], f32, name="stE", tag="st",
                              bufs=2)
                stO = ps.tile([TT, 4 * TT], f32, name="stO", tag="st",
                              bufs=2)
                nc.tensor.matmul(stE[:], ma[0:5, :], mb4[0:5, :],
                                 start=True, stop=False,
                                 skip_group_check=True)
                nc.tensor.matmul(stO[:], ma[64:69, :], mb4[64:69, :],
                                 start=True, stop=False,
                                 skip_group_check=True)
                for i in range(4):
                    c = slice(i * TT, (i + 1) * TT)
                    # head 2i: ft=i rows 0-63; head 2i+1: ft=i rows 64-127
                    nc.tensor.matmul(stE[:, c], kt_[i][0:64, tsl],
                                     qt[i][0:64, tsl],
                                     start=False, stop=(i == 3),
                                     skip_group_check=True)
                    nc.tensor.matmul(stO[:, c], kt_[i][64:128, tsl],
                                     qt[i][64:128, tsl],
                                     start=False, stop=(i == 3),
                                     skip_group_check=True)
                etE = sp.tile([TT, 4 * TT], pv_dt, name="etE", tag="et",
                              bufs=4)
                etO = sp.tile([TT, 4 * TT], pv_dt, name="etO", tag="et",
                              bufs=4)
                nc.scalar.activation(etE[:], stE[:], AF.Exp)
                nc.scalar.activation(etO[:], stO[:], AF.Exp)

                # PV: 2-bank psum [100, 2x512]; head h at bank h//4,
                # col (h%4)*65 (65 cols incl denominator)
                pv = ps.tile([TT, 1024], f32, name="pv", tag="pv", bufs=1)
                for h in range(H):
                    et = etE if h % 2 == 0 else etO
                    blk = h // 2
                    off = (h // 4) * 512 + (h % 4) * 65
                    nc.tensor.matmul(pv[:, off:off + 65],
                                     et[:, blk * TT:(blk + 1) * TT],
                                     vt[t][:, h * 65:(h + 1) * 65],
                                     start=True, stop=True,
                                     skip_group_check=True)
                pvb = pv.rearrange("p (b s) -> p b s", s=512)
                pvq = pvb[:, :, 0:4 * 65].rearrange("p b (q c) -> p b q c",
                                                    c=65)
                rc = sp.tile([TT, 8], f32, name="rc", tag="rc", bufs=4)
                rcv = rc.rearrange("p (b q c) -> p b q c", b=2, c=1)
                nc.vector.reciprocal(rcv, pvq[:, :, :, D:D + 1])
                ov = ot[t].rearrange("p (b q c) -> p b q c", b=2, c=D)
                i0, i1 = broadcast_tensor_aps(pvq[:, :, :, 0:D], rcv)
                nc.vector.tensor_tensor(ov, i0, i1, op=ALU.mult)
                nc.gpsimd.dma_start(out=out_d[tsl, :], in_=ot[t][:])

            # ---- pipelined emission ----
            for ft in range(4):
                qk_group(wq, bqc, qt, ft, 0)
            for ft in range(4):
                qk_group(wk, bkc, kt_, ft, 0)
            v_tile(0)
            v_tile(1)
            v_tile(2)
            att_tile(0)
            v_tile(3)
            att_tile(1)
            qk_group(wq, bqc, qt, 0, 1)
            att_tile(2)
            qk_group(wq, bqc, qt, 1, 1)
            att_tile(3)
            qk_group(wq, bqc, qt, 2, 1)
            qk_group(wq, bqc, qt, 3, 1)
            for ft in range(4):
                qk_group(wk, bkc, kt_, ft, 1)
            v_tile(4)
            v_tile(5)
            v_tile(6)
            att_tile(4)
            v_tile(7)
            att_tile(5)
            att_tile(6)
            att_tile(7)

    nc.compile()
    return nc


def _prep_inputs(x, Wq, bq, Wk, bk, Wv, bv, cfg):
    import ml_dtypes

    x = np.asarray(x, np.float32)
    Wq = np.asarray(Wq, np.float32)
    bq = np.asarray(bq, np.float32)
    Wk = np.asarray(Wk, np.float32)
    bk = np.asarray(bk, np.float32)
    Wv = np.asarray(Wv, np.float32)
    bv = np.asarray(bv, np.float32)

    scale = 1.0 / np.sqrt(np.float32(D))  # 1/8, exact
    wq_s = (Wq * scale).astype(np.float32)
    bq_s = (bq * scale).astype(np.float32)

    io_np = {"bf16": ml_dtypes.bfloat16,
             "f16": np.float16}.get(cfg["proj"], np.float32)
    mask_np = np.float16 if cfg["qk"] == "f16" else ml_dtypes.bfloat16
    xT = np.ascontiguousarray(x.transpose(0, 2, 1))  # [B, DIN, N]

    bqc = np.ascontiguousarray(bq_s.reshape(4, 128).T)
    bkc = np.ascontiguousarray(bk.reshape(4, 128).T)
    bvb = np.ascontiguousarray(np.tile(bv[None, :], (128, 1)))

    # rank-5 factors of the additive frame mask over one 100-token tile,
    # replicated at partition rows 0-4 (even-head bank) and 64-68 (odd)
    big = mask_np(NEGB)
    mA = np.zeros((128, TT), mask_np)
    mB = np.zeros((128, TT), mask_np)
    for base in (0, 64):
        mA[base, :] = 1
        mB[base, :] = -big
        for f in range(4):
            mA[base + 1 + f, f * JN:(f + 1) * JN] = 1
            mB[base + 1 + f, f * JN:(f + 1) * JN] = big
    mB4 = np.ascontiguousarray(np.tile(mB, (1, 4)))

    in_maps = []
    for c in range(NCORES):
        b, fb = c // 4, c % 4
        in_maps.append({
            "xT": np.ascontiguousarray(
                xT[b, :, fb * TOK:(fb + 1) * TOK]).astype(io_np),
            "wq": wq_s.astype(io_np),
            "wk": Wk.astype(io_np),
            "wv": Wv.astype(io_np),
            "bqc": bqc, "bkc": bkc, "bvb": bvb,
            "mA": mA, "mB4": mB4,
        })
    return in_maps


def kernel(x, Wq, bq, Wk, bk, Wv, bv, att_heads=H, latent_dim=D,
           time_len=TL, joint_num=JN, **_):
    from concourse.bass_utils import run_bass_kernel_spmd

    cfg = tuple(sorted(CONFIG.items()))
    if cfg not in _CACHE:
        _CACHE[cfg] = _build(CONFIG)
    nc = _CACHE[cfg]

    in_maps = _prep_inputs(x, Wq, bq, Wk, bk, Wv, bv, CONFIG)
    res = run_bass_kernel_spmd(nc, in_maps, core_ids=list(range(NCORES)))
    global LAST_RESULT
    LAST_RESULT = res

    out = np.empty((B, N, DIN), np.float32)
    for c in range(NCORES):
        b, fb = c // 4, c % 4
        out[b, fb * TOK:(fb + 1) * TOK, :] = res.results[c]["out"]
    return out


# revision 12
# speedup vs baseline: 1.3794x; 1.0461x over previous
"""Block-diagonal (per-frame) multi-head attention on 8 Trainium2 cores.

Problem: x[2,3200,512] -> QKV proj (H=8 heads, D=64) -> attention masked to
25-token frames (128 frames) -> out[2,3200,512].  N = 3200 = 128*25.

Sharding: 256 (batch, frame) groups; core c handles batch c//4, frames
(c%4)*32..+32  => 800 tokens/core, tiled as 8 x 100 tokens (4 frames).

v2 layout/schedule:
  - All stages in ONE pool scope so the Tile scheduler can overlap the
    QKV projections with attention tiles (no phase barrier).
  - Projections contract over the partition dim: qT/kT [feat, tok] =
    W.T @ xT with W-slices stationary; v [tok, feat] = xT.T @ Wv.
  - Per 100-token tile, scores live in TWO psum banks: stE [100, 4*100]
    holds the 4 even heads (PE rows 0-63), stO the odd heads (rows
    64-127) -- separate banks so the PE's row-group-concurrent matmuls
    never co-write a bank.  A rank-5 mask matmul (f16-safe +-30000)
    initializes each bank; exp is ONE activation per bank.
  - v has a ones-column per head so PV's last column yields the softmax
    denominator; per tile ONE reciprocal + ONE broadcast multiply
    produce the normalized output.
  - Outputs DMA on the gpsimd queue so they don't head-block inputs.
"""

import numpy as np

B, N, DIN = 2, 3200, 512
H, D = 8, 64
TL, JN = 128, 25
NCORES = 8
TOK = 800      # tokens per core
NT = 8         # token tiles per core
TT = 100       # tokens per tile (4 frames)
CH = 400       # proj column-chunk (2 chunks)
NEGB = 30000.0  # additive mask magnitude (f16-safe; |scores| <~ 10)

# matmul dtype per stage: 'f32' | 'f32r' | 'bf16' | 'f16'
CONFIG = {"proj": "f16", "qk": "f16", "pv": "f16"}
NWARM = 32     # PE-warmup filler matmuls during the input-DMA lead-in

_CACHE = {}
LAST_RESULT = None  # BassKernelResults of the most recent kernel() call


def _build(cfg):
    import concourse.bacc as bacc
    import concourse.tile as tile
    from concourse import mybir
    from concourse.bass import broadcast_tensor_aps

    f32 = mybir.dt.float32
    bf16 = mybir.dt.bfloat16
    f16 = mybir.dt.float16
    f32r = mybir.dt.float32r
    AF = mybir.ActivationFunctionType
    ALU = mybir.AluOpType

    def io_dt(kind):
        return {"f32": f32, "f32r": f32r, "bf16": bf16, "f16": f16}[kind]

    proj_dt = io_dt(cfg["proj"])
    qk_dt = io_dt(cfg["qk"])
    pv_dt = io_dt(cfg["pv"])
    mask_dt = f16 if cfg["qk"] == "f16" else bf16

    nc = bacc.Bacc("TRN2", target_bir_lowering=False, debug=False,
                   num_devices=NCORES)

    # packed layouts: k-slices side by side so every DMA row is >=2KB
    xt_d = nc.dram_tensor("xTp", [128, 4 * TOK], proj_dt,
                          kind="ExternalInput").ap()
    w_d = {}
    for nm in ("wq", "wk", "wv"):
        w_d[nm] = nc.dram_tensor(nm, [128, 4 * DIN], proj_dt,
                                 kind="ExternalInput").ap()
    bqc_d = nc.dram_tensor("bqc", [128, 4], f32, kind="ExternalInput").ap()
    bkc_d = nc.dram_tensor("bkc", [128, 4], f32, kind="ExternalInput").ap()
    bvb_d = nc.dram_tensor("bvb", [128, DIN], f32, kind="ExternalInput").ap()
    ma_d = nc.dram_tensor("mA", [128, TT], mask_dt, kind="ExternalInput").ap()
    mb4_d = nc.dram_tensor("mB4", [128, 4 * TT], mask_dt,
                           kind="ExternalInput").ap()
    out_d = nc.dram_tensor("out", [TOK, DIN], f32, kind="ExternalOutput").ap()

    with tile.TileContext(nc) as tc:
        with (
            tc.tile_pool(name="pp", bufs=1) as pp,
            tc.tile_pool(name="sp", bufs=4) as sp,
            tc.tile_pool(name="ps", bufs=2, space="PSUM") as ps,
        ):
            # ---- persistent tiles (packed: k-slices side by side) ----
            wq_all = pp.tile([128, 4 * DIN], proj_dt, name="wq_all",
                             tag="wq_all")
            wk_all = pp.tile([128, 4 * DIN], proj_dt, name="wk_all",
                             tag="wk_all")
            wv_all = pp.tile([128, 4 * DIN], proj_dt, name="wv_all",
                             tag="wv_all")
            xt_all = pp.tile([128, 4 * TOK], proj_dt, name="xt_all",
                             tag="xt_all")
            wq = [wq_all[:, k * DIN:(k + 1) * DIN] for k in range(4)]
            wk = [wk_all[:, k * DIN:(k + 1) * DIN] for k in range(4)]
            wv = [wv_all[:, k * DIN:(k + 1) * DIN] for k in range(4)]
            xt = [xt_all[:, k * TOK:(k + 1) * TOK] for k in range(4)]
            bqc = pp.tile([128, 4], f32, name="bqc", tag="bqc")
            bkc = pp.tile([128, 4], f32, name="bkc", tag="bkc")
            bvb = pp.tile([128, DIN], f32, name="bvb", tag="bvb")
            ma = pp.tile([128, TT], mask_dt, name="ma", tag="ma")
            mb4 = pp.tile([128, 4 * TT], mask_dt, name="mb4", tag="mb4")

            qt = [pp.tile([128, TOK], qk_dt, name=f"qt{k}", tag=f"qt{k}")
                  for k in range(4)]
            kt_ = [pp.tile([128, TOK], qk_dt, name=f"kt{k}", tag=f"kt{k}")
                   for k in range(4)]
            # v with 65 columns per head: col h*65+64 is all-ones so the PV
            # matmul also produces the softmax denominator in its last column
            vt = [pp.tile([TT, H * (D + 1)], pv_dt, name=f"vt{t}",
                          tag=f"vt{t}") for t in range(NT)]
            ot = [pp.tile([TT, DIN], f32, name=f"ot{t}", tag=f"ot{t}")
                  for t in range(NT)]

            # ---- PE warm-up: junk matmuls keep the PE HAM-busy from t~0
            # so the clock is at 8/8 when real work arrives.  They write a
            # psum slot ('pv' tag) whose first real use is ~15us in, and
            # read a memset tile, so they gate nothing.
            junk = pp.tile([128, 256], qk_dt, name="junk", tag="junk")
            nc.vector.memset(junk[:], 0.0)
            wacc = ps.tile([TT, 1024], f32, name="wacc", tag="pv", bufs=1)
            for i in range(NWARM):
                nc.tensor.matmul(wacc[:, 0:128], junk[:, 0:TT],
                                 junk[:, 0:128], start=True, stop=True,
                                 skip_group_check=True)

            # ---- input DMAs in priority order (one hw queue, in-order) ----
            nc.sync.dma_start(out=wq_all, in_=w_d["wq"])
            nc.sync.dma_start(out=xt_all, in_=xt_d)
            nc.sync.dma_start(out=bqc, in_=bqc_d)
            nc.sync.dma_start(out=bkc, in_=bkc_d)
            nc.sync.dma_start(out=wk_all, in_=w_d["wk"])
            nc.sync.dma_start(out=wv_all, in_=w_d["wv"])
            nc.sync.dma_start(out=bvb, in_=bvb_d)
            nc.sync.dma_start(out=ma, in_=ma_d)
            nc.sync.dma_start(out=mb4, in_=mb4_d)

            # ---- stage emitters ----
            def qk_group(w, bc, dst, ft, ch):
                fsl = slice(ft * 128, (ft + 1) * 128)
                csl = slice(ch * CH, (ch + 1) * CH)
                acc = ps.tile([128, CH], f32, name="acc", tag="acc", bufs=2)
                for k in range(4):
                    nc.tensor.matmul(acc[:], w[k][:, fsl], xt[k][:, csl],
                                     start=(k == 0), stop=(k == 3))
                nc.scalar.activation(dst[ft][:, csl], acc[:], AF.Identity,
                                     bias=bc[:, ft:ft + 1])

            def v_tile(t):
                tsl = slice(t * TT, (t + 1) * TT)
                acc = ps.tile([TT, DIN], f32, name="vacc", tag="vacc", bufs=2)
                for k in range(4):
                    nc.tensor.matmul(acc[:], xt[k][:, tsl], wv[k][:],
                                     start=(k == 0), stop=(k == 3))
                vv = vt[t].rearrange("p (h c) -> p h c", c=D + 1)
                av = acc.rearrange("p (h c) -> p h c", c=D)
                bv = bvb[:TT, :].rearrange("p (h c) -> p h c", c=D)
                nc.vector.scalar_tensor_tensor(vv[:, :, :D], av, 0.0, bv,
                                               op0=ALU.add, op1=ALU.add)
                nc.vector.tensor_scalar_max(vv[:, :, :D], vv[:, :, :D], 0.0)
                nc.vector.memset(vv[:, :, D:D + 1], 1.0)

            def att_tile(t):
                tsl = slice(t * TT, (t + 1) * TT)
                # two banks: even heads (PE rows 0-63) / odd heads (64-127)
                stE = ps.tile([TT, 4 * TT], f32, name="stE", tag="st",
                              bufs=2)
                stO = ps.tile([TT, 4 * TT], f32, name="stO", tag="st",
                              bufs=2)
                nc.tensor.matmul(stE[:], ma[0:5, :], mb4[0:5, :],
                                 start=True, stop=False,
                                 skip_group_check=True)
                nc.tensor.matmul(stO[:], ma[64:69, :], mb4[64:69, :],
                                 start=True, stop=False,
                                 skip_group_check=True)
                for i in range(4):
                    c = slice(i * TT, (i + 1) * TT)
                    # head 2i: ft=i rows 0-63; head 2i+1: ft=i rows 64-127
                    nc.tensor.matmul(stE[:, c], kt_[i][0:64, tsl],
                                     qt[i][0:64, tsl],
                                     start=False, stop=(i == 3),
                                     skip_group_check=True)
                    nc.tensor.matmul(stO[:, c], kt_[i][64:128, tsl],
                                     qt[i][64:128, tsl],
                                     start=False, stop=(i == 3),
                                     skip_group_check=True)
                etE = sp.tile([TT, 4 * TT], pv_dt, name="etE", tag="et",
                              bufs=4)
                etO = sp.tile([TT, 4 * TT], pv_dt, name="etO", tag="et",
                              bufs=4)
                nc.scalar.activation(etE[:], stE[:], AF.Exp)
                nc.scalar.activation(etO[:], stO[:], AF.Exp)

                # PV: 2-bank psum [100, 2x512]; head h at bank h//4,
                # col (h%4)*65 (65 cols incl denominator)
                pv = ps.tile([TT, 1024], f32, name="pv", tag="pv", bufs=1)
                for h in range(H):
                    et = etE if h % 2 == 0 else etO
                    blk = h // 2
                    off = (h // 4) * 512 + (h % 4) * 65
                    nc.tensor.matmul(pv[:, off:off + 65],
                                     et[:, blk * TT:(blk + 1) * TT],
                                     vt[t][:, h * 65:(h + 1) * 65],
                                     start=True, stop=True,
                                     skip_group_check=True)
                pvb = pv.rearrange("p (b s) -> p b s", s=512)
                pvq = pvb[:, :, 0:4 * 65].rearrange("p b (q c) -> p b q c",
                                                    c=65)
                rc = sp.tile([TT, 8], f32, name="rc", tag="rc", bufs=4)
                rcv = rc.rearrange("p (b q c) -> p b q c", b=2, c=1)
                nc.vector.reciprocal(rcv, pvq[:, :, :, D:D + 1])
                ov = ot[t].rearrange("p (b q c) -> p b q c", b=2, c=D)
                i0, i1 = broadcast_tensor_aps(pvq[:, :, :, 0:D], rcv)
                nc.vector.tensor_tensor(ov, i0, i1, op=ALU.mult)
                nc.gpsimd.dma_start(out=out_d[tsl, :], in_=ot[t][:])

            # ---- pipelined emission ----
            for ft in range(4):
                qk_group(wq, bqc, qt, ft, 0)
            for ft in range(4):
                qk_group(wk, bkc, kt_, ft, 0)
            v_tile(0)
            v_tile(1)
            v_tile(2)
            att_tile(0)
            v_tile(3)
            att_tile(1)
            qk_group(wq, bqc, qt, 0, 1)
            att_tile(2)
            qk_group(wq, bqc, qt, 1, 1)
            att_tile(3)
            qk_group(wq, bqc, qt, 2, 1)
            qk_group(wq, bqc, qt, 3, 1)
            for ft in range(4):
                qk_group(wk, bkc, kt_, ft, 1)
            v_tile(4)
            v_tile(5)
            v_tile(6)
            att_tile(4)
            v_tile(7)
            att_tile(5)
            att_tile(6)
            att_tile(7)

    nc.compile()
    return nc


def _prep_inputs(x, Wq, bq, Wk, bk, Wv, bv, cfg):
    import ml_dtypes

    x = np.asarray(x, np.float32)
    Wq = np.asarray(Wq, np.float32)
    bq = np.asarray(bq, np.float32)
    Wk = np.asarray(Wk, np.float32)
    bk = np.asarray(bk, np.float32)
    Wv = np.asarray(Wv, np.float32)
    bv = np.asarray(bv, np.float32)

    scale = 1.0 / np.sqrt(np.float32(D))  # 1/8, exact
    wq_s = (Wq * scale).astype(np.float32)
    bq_s = (bq * scale).astype(np.float32)

    io_np = {"bf16": ml_dtypes.bfloat16,
             "f16": np.float16}.get(cfg["proj"], np.float32)
    mask_np = np.float16 if cfg["qk"] == "f16" else ml_dtypes.bfloat16
    xT = np.ascontiguousarray(x.transpose(0, 2, 1))  # [B, DIN, N]

    bqc = np.ascontiguousarray(bq_s.reshape(4, 128).T)
    bkc = np.ascontiguousarray(bk.reshape(4, 128).T)
    bvb = np.ascontiguousarray(np.tile(bv[None, :], (128, 1)))

    # rank-5 factors of the additive frame mask over one 100-token tile,
    # replicated at partition rows 0-4 (even-head bank) and 64-68 (odd)
    big = mask_np(NEGB)
    mA = np.zeros((128, TT), mask_np)
    mB = np.zeros((128, TT), mask_np)
    for base in (0, 64):
        mA[base, :] = 1
        mB[base, :] = -big
        for f in range(4):
            mA[base + 1 + f, f * JN:(f + 1) * JN] = 1
            mB[base + 1 + f, f * JN:(f + 1) * JN] = big
    mB4 = np.ascontiguousarray(np.tile(mB, (1, 4)))

    def pack_w(w):
        # [512, 512] -> [128, 4*512]: k-slices side by side (2KB+ DMA rows)
        return np.ascontiguousarray(
            w.reshape(4, 128, DIN).transpose(1, 0, 2).reshape(128, 4 * DIN)
        ).astype(io_np)

    wq_p, wk_p, wv_p = pack_w(wq_s), pack_w(Wk), pack_w(Wv)

    in_maps = []
    for c in range(NCORES):
        b, fb = c // 4, c % 4
        xc = xT[b, :, fb * TOK:(fb + 1) * TOK]  # [512, 800]
        xt_p = np.ascontiguousarray(
            xc.reshape(4, 128, TOK).transpose(1, 0, 2).reshape(128, 4 * TOK)
        ).astype(io_np)
        in_maps.append({
            "xTp": xt_p,
            "wq": wq_p,
            "wk": wk_p,
            "wv": wv_p,
            "bqc": bqc, "bkc": bkc, "bvb": bvb,
            "mA": mA, "mB4": mB4,
        })
    return in_maps


def kernel(x, Wq, bq, Wk, bk, Wv, bv, att_heads=H, latent_dim=D,
           time_len=TL, joint_num=JN, **_):
    from concourse.bass_utils import run_bass_kernel_spmd

    cfg = tuple(sorted(CONFIG.items()))
    if cfg not in _CACHE:
        _CACHE[cfg] = _build(CONFIG)
    nc = _CACHE[cfg]

    in_maps = _prep_inputs(x, Wq, bq, Wk, bk, Wv, bv, CONFIG)
    res = run_bass_kernel_spmd(nc, in_maps, core_ids=list(range(NCORES)))
    global LAST_RESULT
    LAST_RESULT = res

    out = np.empty((B, N, DIN), np.float32)
    for c in range(NCORES):
        b, fb = c // 4, c % 4
        out[b, fb * TOK:(fb + 1) * TOK, :] = res.results[c]["out"]
    return out


# revision 20
# speedup vs baseline: 1.3869x; 1.0055x over previous
"""Block-diagonal (per-frame) multi-head attention on 8 Trainium2 cores.

Problem: x[2,3200,512] -> QKV proj (H=8 heads, D=64) -> attention masked to
25-token frames (128 frames) -> out[2,3200,512].  N = 3200 = 128*25.

Sharding: 256 (batch, frame) groups; core c handles batch c//4, frames
(c%4)*32..+32  => 800 tokens/core, tiled as 8 x 100 tokens (4 frames).

v2 layout/schedule:
  - All stages in ONE pool scope so the Tile scheduler can overlap the
    QKV projections with attention tiles (no phase barrier).
  - Projections contract over the partition dim: qT/kT [feat, tok] =
    W.T @ xT with W-slices stationary; v [tok, feat] = xT.T @ Wv.
  - Per 100-token tile, scores live in TWO psum banks: stE [100, 4*100]
    holds the 4 even heads (PE rows 0-63), stO the odd heads (rows
    64-127) -- separate banks so the PE's row-group-concurrent matmuls
    never co-write a bank.  A rank-5 mask matmul (f16-safe +-30000)
    initializes each bank; exp is ONE activation per bank.
  - v has a ones-column per head so PV's last column yields the softmax
    denominator; per tile ONE reciprocal + ONE broadcast multiply
    produce the normalized output.
  - Outputs DMA on the gpsimd queue so they don't head-block inputs.
"""

import numpy as np

B, N, DIN = 2, 3200, 512
H, D = 8, 64
TL, JN = 128, 25
NCORES = 8
TOK = 800      # tokens per core
NT = 8         # token tiles per core
TT = 100       # tokens per tile (4 frames)
CH = 400       # proj column-chunk (2 chunks)
NEGB = 30000.0  # additive mask magnitude (f16-safe; |scores| <~ 10)

# matmul dtype per stage: 'f32' | 'f32r' | 'bf16' | 'f16'
CONFIG = {"proj": "f16", "qk": "f16", "pv": "f16"}
NWARM = 80     # PE-warmup filler matmuls during the input-DMA lead-in

_CACHE = {}
LAST_RESULT = None  # BassKernelResults of the most recent kernel() call


def _build(cfg):
    import concourse.bacc as bacc
    import concourse.tile as tile
    from concourse import mybir
    from concourse.bass import broadcast_tensor_aps

    f32 = mybir.dt.float32
    bf16 = mybir.dt.bfloat16
    f16 = mybir.dt.float16
    f32r = mybir.dt.float32r
    AF = mybir.ActivationFunctionType
    ALU = mybir.AluOpType

    def io_dt(kind):
        return {"f32": f32, "f32r": f32r, "bf16": bf16, "f16": f16}[kind]

    proj_dt = io_dt(cfg["proj"])
    qk_dt = io_dt(cfg["qk"])
    pv_dt = io_dt(cfg["pv"])
    mask_dt = f16 if cfg["qk"] == "f16" else bf16

    nc = bacc.Bacc("TRN2", target_bir_lowering=False, debug=False,
                   num_devices=NCORES)

    # packed layouts: k-slices side by side so every DMA row is >=2KB
    xt_d = nc.dram_tensor("xTp", [128, 4 * TOK], proj_dt,
                          kind="ExternalInput").ap()
    w_d = {}
    for nm in ("wq", "wk", "wv"):
        w_d[nm] = nc.dram_tensor(nm, [128, 4 * DIN], proj_dt,
                                 kind="ExternalInput").ap()
    bqc_d = nc.dram_tensor("bqc", [128, 4], f32, kind="ExternalInput").ap()
    bkc_d = nc.dram_tensor("bkc", [128, 4], f32, kind="ExternalInput").ap()
    bvr_d = nc.dram_tensor("bvr", [1, DIN], f32, kind="ExternalInput").ap()
    ma_d = nc.dram_tensor("mA", [5, TT], mask_dt, kind="ExternalInput").ap()
    mb4_d = nc.dram_tensor("mB4", [5, 4 * TT], mask_dt,
                           kind="ExternalInput").ap()
    out_d = nc.dram_tensor("out", [TOK, DIN], f32, kind="ExternalOutput").ap()

    with tile.TileContext(nc) as tc:
        with (
            tc.tile_pool(name="pp", bufs=1) as pp,
            tc.tile_pool(name="sp", bufs=4) as sp,
            tc.tile_pool(name="ps", bufs=2, space="PSUM") as ps,
        ):
            # ---- persistent tiles (packed: k-slices side by side) ----
            wq_all = pp.tile([128, 4 * DIN], proj_dt, name="wq_all",
                             tag="wq_all")
            wk_all = pp.tile([128, 4 * DIN], proj_dt, name="wk_all",
                             tag="wk_all")
            wv_all = pp.tile([128, 4 * DIN], proj_dt, name="wv_all",
                             tag="wv_all")
            xt_all = pp.tile([128, 4 * TOK], proj_dt, name="xt_all",
                             tag="xt_all")
            wq = [wq_all[:, k * DIN:(k + 1) * DIN] for k in range(4)]
            wk = [wk_all[:, k * DIN:(k + 1) * DIN] for k in range(4)]
            wv = [wv_all[:, k * DIN:(k + 1) * DIN] for k in range(4)]
            xt = [xt_all[:, k * TOK:(k + 1) * TOK] for k in range(4)]
            bqc = pp.tile([128, 4], f32, name="bqc", tag="bqc")
            bkc = pp.tile([128, 4], f32, name="bkc", tag="bkc")
            bvb = pp.tile([128, DIN], f32, name="bvb", tag="bvb")
            ma = pp.tile([128, TT], mask_dt, name="ma", tag="ma")
            mb4 = pp.tile([128, 4 * TT], mask_dt, name="mb4", tag="mb4")

            qt = [pp.tile([128, TOK], qk_dt, name=f"qt{k}", tag=f"qt{k}")
                  for k in range(4)]
            kt_ = [pp.tile([128, TOK], qk_dt, name=f"kt{k}", tag=f"kt{k}")
                   for k in range(4)]
            # v with 65 columns per head: col h*65+64 is all-ones so the PV
            # matmul also produces the softmax denominator in its last column
            vt = [pp.tile([TT, H * (D + 1)], pv_dt, name=f"vt{t}",
                          tag=f"vt{t}") for t in range(NT)]
            ot = [pp.tile([TT, DIN], f32, name=f"ot{t}", tag=f"ot{t}")
                  for t in range(NT)]

            # ---- PE warm-up: junk matmuls keep the PE HAM-busy from t~0
            # so the clock is at 8/8 when real work arrives.  They write a
            # psum slot ('pv' tag) whose first real use is ~15us in, and
            # read a memset tile, so they gate nothing.
            junk = pp.tile([128, 256], qk_dt, name="junk", tag="junk")
            nc.vector.memset(junk[:], 0.0)
            wacc = ps.tile([TT, 1024], f32, name="wacc", tag="pv", bufs=1)
            for i in range(NWARM):
                nc.tensor.matmul(wacc[:, 0:128], junk[:, 0:TT],
                                 junk[:, 0:128], start=True, stop=True,
                                 skip_group_check=True)

            # ---- input DMAs: two hw queues stream concurrently; each
            # queue is in priority order.  Weights on sync, x + small
            # tensors on scalar, so the critical (wq, xt) pair shares the
            # full HBM port instead of serializing on one queue.
            nc.sync.dma_start(out=wq_all, in_=w_d["wq"])
            nc.sync.dma_start(out=wk_all, in_=w_d["wk"])
            nc.sync.dma_start(out=wv_all, in_=w_d["wv"])
            nc.scalar.dma_start(out=xt_all, in_=xt_d)
            nc.scalar.dma_start(out=bqc, in_=bqc_d)
            nc.scalar.dma_start(out=bkc, in_=bkc_d)
            nc.scalar.dma_start(out=ma[0:5, :], in_=ma_d)
            nc.scalar.dma_start(out=ma[64:69, :], in_=ma_d)
            nc.scalar.dma_start(out=mb4[0:5, :], in_=mb4_d)
            nc.scalar.dma_start(out=mb4[64:69, :], in_=mb4_d)
            # broadcast the v-bias row to 100 partitions straight from DRAM
            # (stride-0 partition dim on the source AP)
            bv_src, _ = broadcast_tensor_aps(bvr_d, bvb[0:TT, :])
            nc.scalar.dma_start(out=bvb[0:TT, :], in_=bv_src)

            # ---- stage emitters ----
            def qk_group(w, bc, dst, ft, ch):
                fsl = slice(ft * 128, (ft + 1) * 128)
                csl = slice(ch * CH, (ch + 1) * CH)
                acc = ps.tile([128, CH], f32, name="acc", tag="acc", bufs=2)
                for k in range(4):
                    nc.tensor.matmul(acc[:], w[k][:, fsl], xt[k][:, csl],
                                     start=(k == 0), stop=(k == 3))
                nc.scalar.activation(dst[ft][:, csl], acc[:], AF.Identity,
                                     bias=bc[:, ft:ft + 1])

            def v_tile(t):
                tsl = slice(t * TT, (t + 1) * TT)
                acc = ps.tile([TT, DIN], f32, name="vacc", tag="vacc", bufs=1)
                for k in range(4):
                    nc.tensor.matmul(acc[:], xt[k][:, tsl], wv[k][:],
                                     start=(k == 0), stop=(k == 3))
                vv = vt[t].rearrange("p (h c) -> p h c", c=D + 1)
                av = acc.rearrange("p (h c) -> p h c", c=D)
                bv = bvb[:TT, :].rearrange("p (h c) -> p h c", c=D)
                nc.vector.scalar_tensor_tensor(vv[:, :, :D], av, 0.0, bv,
                                               op0=ALU.add, op1=ALU.add)
                nc.vector.tensor_scalar_max(vv[:, :, :D], vv[:, :, :D], 0.0)
                nc.vector.memset(vv[:, :, D:D + 1], 1.0)

            def att_tile(t):
                tsl = slice(t * TT, (t + 1) * TT)
                # two banks: even heads (PE rows 0-63) / odd heads (64-127)
                stE = ps.tile([TT, 4 * TT], f32, name="stE", tag="st",
                              bufs=3)
                stO = ps.tile([TT, 4 * TT], f32, name="stO", tag="st",
                              bufs=3)
                nc.tensor.matmul(stE[:], ma[0:5, :], mb4[0:5, :],
                                 start=True, stop=False,
                                 skip_group_check=True)
                nc.tensor.matmul(stO[:], ma[64:69, :], mb4[64:69, :],
                                 start=True, stop=False,
                                 skip_group_check=True)
                for i in range(4):
                    c = slice(i * TT, (i + 1) * TT)
                    # head 2i: ft=i rows 0-63; head 2i+1: ft=i rows 64-127
                    nc.tensor.matmul(stE[:, c], kt_[i][0:64, tsl],
                                     qt[i][0:64, tsl],
                                     start=False, stop=(i == 3),
                                     skip_group_check=True)
                    nc.tensor.matmul(stO[:, c], kt_[i][64:128, tsl],
                                     qt[i][64:128, tsl],
                                     start=False, stop=(i == 3),
                                     skip_group_check=True)
                etE = sp.tile([TT, 4 * TT], pv_dt, name="etE", tag="et",
                              bufs=4)
                etO = sp.tile([TT, 4 * TT], pv_dt, name="etO", tag="et",
                              bufs=4)
                nc.scalar.activation(etE[:], stE[:], AF.Exp)
                nc.scalar.activation(etO[:], stO[:], AF.Exp)

                # PV: 2-bank psum [100, 2x512]; head h at bank h//4,
                # col (h%4)*65 (65 cols incl denominator)
                pv = ps.tile([TT, 1024], f32, name="pv", tag="pv", bufs=1)
                for h in range(H):
                    et = etE if h % 2 == 0 else etO
                    blk = h // 2
                    off = (h // 4) * 512 + (h % 4) * 65
                    nc.tensor.matmul(pv[:, off:off + 65],
                                     et[:, blk * TT:(blk + 1) * TT],
                                     vt[t][:, h * 65:(h + 1) * 65],
                                     start=True, stop=True,
                                     skip_group_check=True)
                pvb = pv.rearrange("p (b s) -> p b s", s=512)
                pvq = pvb[:, :, 0:4 * 65].rearrange("p b (q c) -> p b q c",
                                                    c=65)
                rc = sp.tile([TT, 8], f32, name="rc", tag="rc", bufs=4)
                rcv = rc.rearrange("p (b q c) -> p b q c", b=2, c=1)
                nc.vector.reciprocal(rcv, pvq[:, :, :, D:D + 1])
                ov = ot[t].rearrange("p (b q c) -> p b q c", b=2, c=D)
                i0, i1 = broadcast_tensor_aps(pvq[:, :, :, 0:D], rcv)
                nc.vector.tensor_tensor(ov, i0, i1, op=ALU.mult)
                nc.gpsimd.dma_start(out=out_d[tsl, :], in_=ot[t][:])

            # ---- pipelined emission ----
            for ft in range(4):
                qk_group(wq, bqc, qt, ft, 0)
            for ft in range(4):
                qk_group(wk, bkc, kt_, ft, 0)
            v_tile(0)
            v_tile(1)
            v_tile(2)
            att_tile(0)
            v_tile(3)
            att_tile(1)
            qk_group(wq, bqc, qt, 0, 1)
            att_tile(2)
            qk_group(wq, bqc, qt, 1, 1)
            att_tile(3)
            qk_group(wq, bqc, qt, 2, 1)
            qk_group(wq, bqc, qt, 3, 1)
            for ft in range(4):
                qk_group(wk, bkc, kt_, ft, 1)
            v_tile(4)
            v_tile(5)
            v_tile(6)
            att_tile(4)
            v_tile(7)
            att_tile(5)
            att_tile(6)
            att_tile(7)

    nc.compile()
    return nc


def _prep_inputs(x, Wq, bq, Wk, bk, Wv, bv, cfg):
    import ml_dtypes

    x = np.asarray(x, np.float32)
    Wq = np.asarray(Wq, np.float32)
    bq = np.asarray(bq, np.float32)
    Wk = np.asarray(Wk, np.float32)
    bk = np.asarray(bk, np.float32)
    Wv = np.asarray(Wv, np.float32)
    bv = np.asarray(bv, np.float32)

    scale = 1.0 / np.sqrt(np.float32(D))  # 1/8, exact
    wq_s = (Wq * scale).astype(np.float32)
    bq_s = (bq * scale).astype(np.float32)

    io_np = {"bf16": ml_dtypes.bfloat16,
             "f16": np.float16}.get(cfg["proj"], np.float32)
    mask_np = np.float16 if cfg["qk"] == "f16" else ml_dtypes.bfloat16
    xT = np.ascontiguousarray(x.transpose(0, 2, 1))  # [B, DIN, N]

    bqc = np.ascontiguousarray(bq_s.reshape(4, 128).T)
    bkc = np.ascontiguousarray(bk.reshape(4, 128).T)
    bvr = np.ascontiguousarray(bv[None, :])

    # rank-5 factors of the additive frame mask over one 100-token tile
    # (the kernel DMAs these 5 rows to partition bases 0 and 64)
    big = mask_np(NEGB)
    mA = np.zeros((5, TT), mask_np)
    mB = np.zeros((5, TT), mask_np)
    mA[0, :] = 1
    mB[0, :] = -big
    for f in range(4):
        mA[1 + f, f * JN:(f + 1) * JN] = 1
        mB[1 + f, f * JN:(f + 1) * JN] = big
    mB4 = np.ascontiguousarray(np.tile(mB, (1, 4)))

    def pack_w(w):
        # [512, 512] -> [128, 4*512]: k-slices side by side (2KB+ DMA rows)
        return np.ascontiguousarray(
            w.reshape(4, 128, DIN).transpose(1, 0, 2).reshape(128, 4 * DIN)
        ).astype(io_np)

    wq_p, wk_p, wv_p = pack_w(wq_s), pack_w(Wk), pack_w(Wv)

    in_maps = []
    for c in range(NCORES):
        b, fb = c // 4, c % 4
        xc = xT[b, :, fb * TOK:(fb + 1) * TOK]  # [512, 800]
        xt_p = np.ascontiguousarray(
            xc.reshape(4, 128, TOK).transpose(1, 0, 2).reshape(128, 4 * TOK)
        ).astype(io_np)
        in_maps.append({
            "xTp": xt_p,
            "wq": wq_p,
            "wk": wk_p,
            "wv": wv_p,
            "bqc": bqc, "bkc": bkc, "bvr": bvr,
            "mA": mA, "mB4": mB4,
        })
    return in_maps


def kernel(x, Wq, bq, Wk, bk, Wv, bv, att_heads=H, latent_dim=D,
           time_len=TL, joint_num=JN, **_):
    from concourse.bass_utils import run_bass_kernel_spmd

    cfg = tuple(sorted(CONFIG.items()))
    if cfg not in _CACHE:
        _CACHE[cfg] = _build(CONFIG)
    nc = _CACHE[cfg]

    in_maps = _prep_inputs(x, Wq, bq, Wk, bk, Wv, bv, CONFIG)
    res = run_bass_kernel_spmd(nc, in_maps, core_ids=list(range(NCORES)))
    global LAST_RESULT
    LAST_RESULT = res

    out = np.empty((B, N, DIN), np.float32)
    for c in range(NCORES):
        b, fb = c // 4, c % 4
        out[b, fb * TOK:(fb + 1) * TOK, :] = res.results[c]["out"]
    return out


# revision 22
# speedup vs baseline: 1.4005x; 1.0098x over previous
"""Block-diagonal (per-frame) multi-head attention on 8 Trainium2 cores.

Problem: x[2,3200,512] -> QKV proj (H=8 heads, D=64) -> attention masked to
25-token frames (128 frames) -> out[2,3200,512].  N = 3200 = 128*25.

Sharding: 256 (batch, frame) groups; core c handles batch c//4, frames
(c%4)*32..+32  => 800 tokens/core, tiled as 8 x 100 tokens (4 frames).

v2 layout/schedule:
  - All stages in ONE pool scope so the Tile scheduler can overlap the
    QKV projections with attention tiles (no phase barrier).
  - Projections contract over the partition dim: qT/kT [feat, tok] =
    W.T @ xT with W-slices stationary; v [tok, feat] = xT.T @ Wv.
  - Per 100-token tile, scores live in TWO psum banks: stE [100, 4*100]
    holds the 4 even heads (PE rows 0-63), stO the odd heads (rows
    64-127) -- separate banks so the PE's row-group-concurrent matmuls
    never co-write a bank.  A rank-5 mask matmul (f16-safe +-30000)
    initializes each bank; exp is ONE activation per bank.
  - v has a ones-column per head so PV's last column yields the softmax
    denominator; per tile ONE reciprocal + ONE broadcast multiply
    produce the normalized output.
  - Outputs DMA on the gpsimd queue so they don't head-block inputs.
"""

import numpy as np

B, N, DIN = 2, 3200, 512
H, D = 8, 64
TL, JN = 128, 25
NCORES = 8
TOK = 800      # tokens per core
NT = 8         # token tiles per core
TT = 100       # tokens per tile (4 frames)
CH = 400       # proj column-chunk (2 chunks)
NEGB = 30000.0  # additive mask magnitude (f16-safe; |scores| <~ 10)

# matmul dtype per stage: 'f32' | 'f32r' | 'bf16' | 'f16'
CONFIG = {"proj": "f16", "qk": "f16", "pv": "f16"}
NWARM = 48     # PE-warmup filler matmuls during the input-DMA lead-in

_CACHE = {}
LAST_RESULT = None  # BassKernelResults of the most recent kernel() call


def _build(cfg):
    import concourse.bacc as bacc
    import concourse.tile as tile
    from concourse import mybir
    from concourse.bass import broadcast_tensor_aps

    f32 = mybir.dt.float32
    bf16 = mybir.dt.bfloat16
    f16 = mybir.dt.float16
    f32r = mybir.dt.float32r
    AF = mybir.ActivationFunctionType
    ALU = mybir.AluOpType

    def io_dt(kind):
        return {"f32": f32, "f32r": f32r, "bf16": bf16, "f16": f16}[kind]

    proj_dt = io_dt(cfg["proj"])
    qk_dt = io_dt(cfg["qk"])
    pv_dt = io_dt(cfg["pv"])
    mask_dt = f16 if cfg["qk"] == "f16" else bf16

    nc = bacc.Bacc("TRN2", target_bir_lowering=False, debug=False,
                   num_devices=NCORES)

    # packed layouts: k-slices side by side so every DMA row is >=2KB
    xt_d = nc.dram_tensor("xTp", [128, 4 * TOK], proj_dt,
                          kind="ExternalInput").ap()
    w_d = {}
    for nm in ("wq", "wk", "wv"):
        w_d[nm] = nc.dram_tensor(nm, [128, 4 * DIN], proj_dt,
                                 kind="ExternalInput").ap()
    bqc_d = nc.dram_tensor("bqc", [128, 4], f32, kind="ExternalInput").ap()
    bkc_d = nc.dram_tensor("bkc", [128, 4], f32, kind="ExternalInput").ap()
    bvr_d = nc.dram_tensor("bvr", [1, DIN], f32, kind="ExternalInput").ap()
    ma_d = nc.dram_tensor("mA", [5, TT], mask_dt, kind="ExternalInput").ap()
    mb4_d = nc.dram_tensor("mB4", [5, 4 * TT], mask_dt,
                           kind="ExternalInput").ap()
    out_d = nc.dram_tensor("out", [TOK, DIN], f32, kind="ExternalOutput").ap()

    with tile.TileContext(nc) as tc:
        with (
            tc.tile_pool(name="pp", bufs=1) as pp,
            tc.tile_pool(name="sp", bufs=4) as sp,
            tc.tile_pool(name="ps", bufs=2, space="PSUM") as ps,
        ):
            # ---- persistent tiles (packed: k-slices side by side) ----
            wq_all = pp.tile([128, 4 * DIN], proj_dt, name="wq_all",
                             tag="wq_all")
            wk_all = pp.tile([128, 4 * DIN], proj_dt, name="wk_all",
                             tag="wk_all")
            wv_all = pp.tile([128, 4 * DIN], proj_dt, name="wv_all",
                             tag="wv_all")
            xt_all = pp.tile([128, 4 * TOK], proj_dt, name="xt_all",
                             tag="xt_all")
            wq = [wq_all[:, k * DIN:(k + 1) * DIN] for k in range(4)]
            wk = [wk_all[:, k * DIN:(k + 1) * DIN] for k in range(4)]
            wv = [wv_all[:, k * DIN:(k + 1) * DIN] for k in range(4)]
            xt = [xt_all[:, k * TOK:(k + 1) * TOK] for k in range(4)]
            bqc = pp.tile([128, 4], f32, name="bqc", tag="bqc")
            bkc = pp.tile([128, 4], f32, name="bkc", tag="bkc")
            bvb = pp.tile([128, DIN], f32, name="bvb", tag="bvb")
            ma = pp.tile([128, TT], mask_dt, name="ma", tag="ma")
            mb4 = pp.tile([128, 4 * TT], mask_dt, name="mb4", tag="mb4")

            qt = [pp.tile([128, TOK], qk_dt, name=f"qt{k}", tag=f"qt{k}")
                  for k in range(4)]
            kt_ = [pp.tile([128, TOK], qk_dt, name=f"kt{k}", tag=f"kt{k}")
                   for k in range(4)]
            # v with 65 columns per head: col h*65+64 is all-ones so the PV
            # matmul also produces the softmax denominator in its last column
            vt = [pp.tile([TT, H * (D + 1)], pv_dt, name=f"vt{t}",
                          tag=f"vt{t}") for t in range(NT)]
            ot = [pp.tile([TT, DIN], f32, name=f"ot{t}", tag=f"ot{t}")
                  for t in range(NT)]

            # ---- PE warm-up: junk matmuls keep the PE HAM-busy from t~0
            # so the clock is at 8/8 when real work arrives.  They write a
            # psum slot ('pv' tag) whose first real use is ~15us in, and
            # read a memset tile, so they gate nothing.
            junk = pp.tile([128, 256], qk_dt, name="junk", tag="junk")
            nc.vector.memset(junk[:], 0.0)
            wacc = ps.tile([TT, 1024], f32, name="wacc", tag="pv", bufs=1)
            for i in range(NWARM):
                nc.tensor.matmul(wacc[:, 0:128], junk[:, 0:TT],
                                 junk[:, 0:128], start=True, stop=True,
                                 skip_group_check=True)

            # ---- input DMAs: two hw queues stream concurrently; each
            # queue is in priority order.  Weights on sync, x + small
            # tensors on scalar, so the critical (wq, xt) pair shares the
            # full HBM port instead of serializing on one queue.
            # weights split in halves so the completion semaphores unlock
            # the k=0,1 accumulation matmuls before the full tensor lands
            nc.sync.dma_start(out=wq_all[:, 0:2 * DIN],
                              in_=w_d["wq"][:, 0:2 * DIN])
            nc.sync.dma_start(out=wq_all[:, 2 * DIN:4 * DIN],
                              in_=w_d["wq"][:, 2 * DIN:4 * DIN])
            nc.sync.dma_start(out=wk_all[:, 0:2 * DIN],
                              in_=w_d["wk"][:, 0:2 * DIN])
            nc.sync.dma_start(out=wk_all[:, 2 * DIN:4 * DIN],
                              in_=w_d["wk"][:, 2 * DIN:4 * DIN])
            nc.sync.dma_start(out=wv_all, in_=w_d["wv"])
            # x split by column-chunk: each DMA covers cols [ch*CH, ch*CH+CH)
            # of all 4 k-slices (strided rows of 4x800B -- still big rows)
            xa = xt_all.rearrange("p (k t) -> p k t", t=TOK)
            xd = xt_d.rearrange("p (k t) -> p k t", t=TOK)
            nc.scalar.dma_start(out=xa[:, :, 0:CH], in_=xd[:, :, 0:CH])
            nc.scalar.dma_start(out=xa[:, :, CH:TOK], in_=xd[:, :, CH:TOK])
            nc.scalar.dma_start(out=bqc, in_=bqc_d)
            nc.scalar.dma_start(out=bkc, in_=bkc_d)
            nc.scalar.dma_start(out=ma[0:5, :], in_=ma_d)
            nc.scalar.dma_start(out=ma[64:69, :], in_=ma_d)
            nc.scalar.dma_start(out=mb4[0:5, :], in_=mb4_d)
            nc.scalar.dma_start(out=mb4[64:69, :], in_=mb4_d)
            # broadcast the v-bias row to 100 partitions straight from DRAM
            # (stride-0 partition dim on the source AP)
            bv_src, _ = broadcast_tensor_aps(bvr_d, bvb[0:TT, :])
            nc.scalar.dma_start(out=bvb[0:TT, :], in_=bv_src)

            # ---- stage emitters ----
            def qk_group(w, bc, dst, ft, ch):
                fsl = slice(ft * 128, (ft + 1) * 128)
                csl = slice(ch * CH, (ch + 1) * CH)
                acc = ps.tile([128, CH], f32, name="acc", tag="acc", bufs=2)
                for k in range(4):
                    nc.tensor.matmul(acc[:], w[k][:, fsl], xt[k][:, csl],
                                     start=(k == 0), stop=(k == 3))
                nc.scalar.activation(dst[ft][:, csl], acc[:], AF.Identity,
                                     bias=bc[:, ft:ft + 1])

            def v_tile(t):
                tsl = slice(t * TT, (t + 1) * TT)
                acc = ps.tile([TT, DIN], f32, name="vacc", tag="vacc", bufs=1)
                for k in range(4):
                    nc.tensor.matmul(acc[:], xt[k][:, tsl], wv[k][:],
                                     start=(k == 0), stop=(k == 3))
                vv = vt[t].rearrange("p (h c) -> p h c", c=D + 1)
                av = acc.rearrange("p (h c) -> p h c", c=D)
                bv = bvb[:TT, :].rearrange("p (h c) -> p h c", c=D)
                nc.vector.scalar_tensor_tensor(vv[:, :, :D], av, 0.0, bv,
                                               op0=ALU.add, op1=ALU.add)
                nc.vector.tensor_scalar_max(vv[:, :, :D], vv[:, :, :D], 0.0)
                nc.vector.memset(vv[:, :, D:D + 1], 1.0)

            def att_tile(t):
                tsl = slice(t * TT, (t + 1) * TT)
                # two banks: even heads (PE rows 0-63) / odd heads (64-127)
                stE = ps.tile([TT, 4 * TT], f32, name="stE", tag="st",
                              bufs=3)
                stO = ps.tile([TT, 4 * TT], f32, name="stO", tag="st",
                              bufs=3)
                nc.tensor.matmul(stE[:], ma[0:5, :], mb4[0:5, :],
                                 start=True, stop=False,
                                 skip_group_check=True)
                nc.tensor.matmul(stO[:], ma[64:69, :], mb4[64:69, :],
                                 start=True, stop=False,
                                 skip_group_check=True)
                for i in range(4):
                    c = slice(i * TT, (i + 1) * TT)
                    # head 2i: ft=i rows 0-63; head 2i+1: ft=i rows 64-127
                    nc.tensor.matmul(stE[:, c], kt_[i][0:64, tsl],
                                     qt[i][0:64, tsl],
                                     start=False, stop=(i == 3),
                                     skip_group_check=True)
                    nc.tensor.matmul(stO[:, c], kt_[i][64:128, tsl],
                                     qt[i][64:128, tsl],
                                     start=False, stop=(i == 3),
                                     skip_group_check=True)
                etE = sp.tile([TT, 4 * TT], pv_dt, name="etE", tag="et",
                              bufs=4)
                etO = sp.tile([TT, 4 * TT], pv_dt, name="etO", tag="et",
                              bufs=4)
                nc.scalar.activation(etE[:], stE[:], AF.Exp)
                nc.scalar.activation(etO[:], stO[:], AF.Exp)

                # PV: 2-bank psum [100, 2x512]; head h at bank h//4,
                # col (h%4)*65 (65 cols incl denominator)
                pv = ps.tile([TT, 1024], f32, name="pv", tag="pv", bufs=1)
                for h in range(H):
                    et = etE if h % 2 == 0 else etO
                    blk = h // 2
                    off = (h // 4) * 512 + (h % 4) * 65
                    nc.tensor.matmul(pv[:, off:off + 65],
                                     et[:, blk * TT:(blk + 1) * TT],
                                     vt[t][:, h * 65:(h + 1) * 65],
                                     start=True, stop=True,
                                     skip_group_check=True)
                pvb = pv.rearrange("p (b s) -> p b s", s=512)
                pvq = pvb[:, :, 0:4 * 65].rearrange("p b (q c) -> p b q c",
                                                    c=65)
                rc = sp.tile([TT, 8], f32, name="rc", tag="rc", bufs=4)
                rcv = rc.rearrange("p (b q c) -> p b q c", b=2, c=1)
                nc.vector.reciprocal(rcv, pvq[:, :, :, D:D + 1])
                ov = ot[t].rearrange("p (b q c) -> p b q c", b=2, c=D)
                i0, i1 = broadcast_tensor_aps(pvq[:, :, :, 0:D], rcv)
                nc.vector.tensor_tensor(ov, i0, i1, op=ALU.mult)
                nc.gpsimd.dma_start(out=out_d[tsl, :], in_=ot[t][:])

            # ---- pipelined emission ----
            for ft in range(4):
                qk_group(wq, bqc, qt, ft, 0)
            for ft in range(4):
                qk_group(wk, bkc, kt_, ft, 0)
            v_tile(0)
            v_tile(1)
            v_tile(2)
            att_tile(0)
            v_tile(3)
            att_tile(1)
            qk_group(wq, bqc, qt, 0, 1)
            att_tile(2)
            qk_group(wq, bqc, qt, 1, 1)
            att_tile(3)
            qk_group(wq, bqc, qt, 2, 1)
            qk_group(wq, bqc, qt, 3, 1)
            for ft in range(4):
                qk_group(wk, bkc, kt_, ft, 1)
            v_tile(4)
            v_tile(5)
            v_tile(6)
            att_tile(4)
            v_tile(7)
            att_tile(5)
            att_tile(6)
            att_tile(7)

    nc.compile()
    return nc


def _prep_inputs(x, Wq, bq, Wk, bk, Wv, bv, cfg):
    import ml_dtypes

    x = np.asarray(x, np.float32)
    Wq = np.asarray(Wq, np.float32)
    bq = np.asarray(bq, np.float32)
    Wk = np.asarray(Wk, np.float32)
    bk = np.asarray(bk, np.float32)
    Wv = np.asarray(Wv, np.float32)
    bv = np.asarray(bv, np.float32)

    scale = 1.0 / np.sqrt(np.float32(D))  # 1/8, exact
    wq_s = (Wq * scale).astype(np.float32)
    bq_s = (bq * scale).astype(np.float32)

    io_np = {"bf16": ml_dtypes.bfloat16,
             "f16": np.float16}.get(cfg["proj"], np.float32)
    mask_np = np.float16 if cfg["qk"] == "f16" else ml_dtypes.bfloat16
    xT = np.ascontiguousarray(x.transpose(0, 2, 1))  # [B, DIN, N]

    bqc = np.ascontiguousarray(bq_s.reshape(4, 128).T)
    bkc = np.ascontiguousarray(bk.reshape(4, 128).T)
    bvr = np.ascontiguousarray(bv[None, :])

    # rank-5 factors of the additive frame mask over one 100-token tile
    # (the kernel DMAs these 5 rows to partition bases 0 and 64)
    big = mask_np(NEGB)
    mA = np.zeros((5, TT), mask_np)
    mB = np.zeros((5, TT), mask_np)
    mA[0, :] = 1
    mB[0, :] = -big
    for f in range(4):
        mA[1 + f, f * JN:(f + 1) * JN] = 1
        mB[1 + f, f * JN:(f + 1) * JN] = big
    mB4 = np.ascontiguousarray(np.tile(mB, (1, 4)))

    def pack_w(w):
        # [512, 512] -> [128, 4*512]: k-slices side by side (2KB+ DMA rows)
        return np.ascontiguousarray(
            w.reshape(4, 128, DIN).transpose(1, 0, 2).reshape(128, 4 * DIN)
        ).astype(io_np)

    wq_p, wk_p, wv_p = pack_w(wq_s), pack_w(Wk), pack_w(Wv)

    in_maps = []
    for c in range(NCORES):
        b, fb = c // 4, c % 4
        xc = xT[b, :, fb * TOK:(fb + 1) * TOK]  # [512, 800]
        xt_p = np.ascontiguousarray(
            xc.reshape(4, 128, TOK).transpose(1, 0, 2).reshape(128, 4 * TOK)
        ).astype(io_np)
        in_maps.append({
            "xTp": xt_p,
            "wq": wq_p,
            "wk": wk_p,
            "wv": wv_p,
            "bqc": bqc, "bkc": bkc, "bvr": bvr,
            "mA": mA, "mB4": mB4,
        })
    return in_maps


def kernel(x, Wq, bq, Wk, bk, Wv, bv, att_heads=H, latent_dim=D,
           time_len=TL, joint_num=JN, **_):
    from concourse.bass_utils import run_bass_kernel_spmd

    cfg = tuple(sorted(CONFIG.items()))
    if cfg not in _CACHE:
        _CACHE[cfg] = _build(CONFIG)
    nc = _CACHE[cfg]

    in_maps = _prep_inputs(x, Wq, bq, Wk, bk, Wv, bv, CONFIG)
    res = run_bass_kernel_spmd(nc, in_maps, core_ids=list(range(NCORES)))
    global LAST_RESULT
    LAST_RESULT = res

    out = np.empty((B, N, DIN), np.float32)
    for c in range(NCORES):
        b, fb = c // 4, c % 4
        out[b, fb * TOK:(fb + 1) * TOK, :] = res.results[c]["out"]
    return out


# revision 26
# speedup vs baseline: 1.4581x; 1.0411x over previous
"""Block-diagonal (per-frame) multi-head attention on 8 Trainium2 cores.

Problem: x[2,3200,512] -> QKV proj (H=8 heads, D=64) -> attention masked to
25-token frames (128 frames) -> out[2,3200,512].  N = 3200 = 128*25.

Sharding: 256 (batch, frame) groups; core c handles batch c//4, frames
(c%4)*32..+32  => 800 tokens/core, tiled as 8 x 100 tokens (4 frames).

v2 layout/schedule:
  - All stages in ONE pool scope so the Tile scheduler can overlap the
    QKV projections with attention tiles (no phase barrier).
  - Projections contract over the partition dim: qT/kT [feat, tok] =
    W.T @ xT with W-slices stationary; v [tok, feat] = xT.T @ Wv.
  - Per 100-token tile, scores live in TWO psum banks: stE [100, 4*100]
    holds the 4 even heads (PE rows 0-63), stO the odd heads (rows
    64-127) -- separate banks so the PE's row-group-concurrent matmuls
    never co-write a bank.  A rank-5 mask matmul (f16-safe +-30000)
    initializes each bank; exp is ONE activation per bank.
  - v has a ones-column per head so PV's last column yields the softmax
    denominator; per tile ONE reciprocal + ONE broadcast multiply
    produce the normalized output.
  - Outputs DMA on the gpsimd queue so they don't head-block inputs.
"""

import numpy as np

B, N, DIN = 2, 3200, 512
H, D = 8, 64
TL, JN = 128, 25
NCORES = 8
TOK = 800      # tokens per core
NT = 8         # token tiles per core
TT = 100       # tokens per tile (4 frames)
CH = 400       # proj column-chunk (2 chunks)
NEGB = 30000.0  # additive mask magnitude (f16-safe; |scores| <~ 10)

# matmul dtype per stage: 'f32' | 'f32r' | 'bf16' | 'f16'
CONFIG = {"proj": "f16", "qk": "f16", "pv": "f16"}
NWARM = 48     # PE-warmup filler matmuls during the input-DMA lead-in

_CACHE = {}
LAST_RESULT = None  # BassKernelResults of the most recent kernel() call


def _build(cfg):
    import concourse.bacc as bacc
    import concourse.tile as tile
    from concourse import mybir
    from concourse.bass import broadcast_tensor_aps

    f32 = mybir.dt.float32
    bf16 = mybir.dt.bfloat16
    f16 = mybir.dt.float16
    f32r = mybir.dt.float32r
    AF = mybir.ActivationFunctionType
    ALU = mybir.AluOpType

    def io_dt(kind):
        return {"f32": f32, "f32r": f32r, "bf16": bf16, "f16": f16}[kind]

    proj_dt = io_dt(cfg["proj"])
    qk_dt = io_dt(cfg["qk"])
    pv_dt = io_dt(cfg["pv"])
    mask_dt = f16 if cfg["qk"] == "f16" else bf16

    nc = bacc.Bacc("TRN2", target_bir_lowering=False, debug=False,
                   num_devices=NCORES)

    # packed layouts: k-slices side by side so every DMA row is >=2KB
    xt_d = nc.dram_tensor("xTp", [128, 4 * TOK], proj_dt,
                          kind="ExternalInput").ap()
    w_d = {}
    for nm in ("wq", "wk", "wv"):
        w_d[nm] = nc.dram_tensor(nm, [128, 4 * DIN], proj_dt,
                                 kind="ExternalInput").ap()
    bqc_d = nc.dram_tensor("bqc", [128, 4], f32, kind="ExternalInput").ap()
    bkc_d = nc.dram_tensor("bkc", [128, 4], f32, kind="ExternalInput").ap()
    bvr_d = nc.dram_tensor("bvr", [1, DIN], f32, kind="ExternalInput").ap()
    ma_d = nc.dram_tensor("mA", [5, TT], mask_dt, kind="ExternalInput").ap()
    mb4_d = nc.dram_tensor("mB4", [5, 4 * TT], mask_dt,
                           kind="ExternalInput").ap()
    out_d = nc.dram_tensor("out", [TOK, DIN], f32, kind="ExternalOutput").ap()

    with tile.TileContext(nc) as tc:
        with (
            tc.tile_pool(name="pp", bufs=1) as pp,
            tc.tile_pool(name="sp", bufs=4) as sp,
            tc.tile_pool(name="ps", bufs=2, space="PSUM") as ps,
        ):
            # ---- persistent tiles (packed: k-slices side by side) ----
            wq_all = pp.tile([128, 4 * DIN], proj_dt, name="wq_all",
                             tag="wq_all")
            wk_all = pp.tile([128, 4 * DIN], proj_dt, name="wk_all",
                             tag="wk_all")
            wv_all = pp.tile([128, 4 * DIN], proj_dt, name="wv_all",
                             tag="wv_all")
            xt_all = pp.tile([128, 4 * TOK], proj_dt, name="xt_all",
                             tag="xt_all")
            wq = [wq_all[:, k * DIN:(k + 1) * DIN] for k in range(4)]
            wk = [wk_all[:, k * DIN:(k + 1) * DIN] for k in range(4)]
            wv = [wv_all[:, k * DIN:(k + 1) * DIN] for k in range(4)]
            xt = [xt_all[:, k * TOK:(k + 1) * TOK] for k in range(4)]
            bqc = pp.tile([128, 4], f32, name="bqc", tag="bqc")
            bkc = pp.tile([128, 4], f32, name="bkc", tag="bkc")
            bvb = pp.tile([128, DIN], f32, name="bvb", tag="bvb")
            ma = pp.tile([128, TT], mask_dt, name="ma", tag="ma")
            mb4 = pp.tile([128, 4 * TT], mask_dt, name="mb4", tag="mb4")

            qt = [pp.tile([128, TOK], qk_dt, name=f"qt{k}", tag=f"qt{k}")
                  for k in range(4)]
            kt_ = [pp.tile([128, TOK], qk_dt, name=f"kt{k}", tag=f"kt{k}")
                   for k in range(4)]
            # v with 65 columns per head: col h*65+64 is all-ones so the PV
            # matmul also produces the softmax denominator in its last column
            vt = [pp.tile([TT, H * (D + 1)], pv_dt, name=f"vt{t}",
                          tag=f"vt{t}") for t in range(NT)]
            ot = [pp.tile([TT, DIN], f32, name=f"ot{t}", tag=f"ot{t}")
                  for t in range(NT)]

            # ---- PE warm-up: junk matmuls keep the PE HAM-busy from t~0
            # so the clock is at 8/8 when real work arrives.  They write a
            # psum slot ('pv' tag) whose first real use is ~15us in, and
            # read a memset tile, so they gate nothing.
            junk = pp.tile([128, 256], qk_dt, name="junk", tag="junk")
            nc.vector.memset(junk[:], 0.0)
            wacc = ps.tile([TT, 1024], f32, name="wacc", tag="pv", bufs=1)
            for i in range(NWARM):
                nc.tensor.matmul(wacc[:, 0:128], junk[:, 0:TT],
                                 junk[:, 0:128], start=True, stop=True,
                                 skip_group_check=True)

            # ---- input DMAs: two hw queues stream concurrently; each
            # queue is in priority order.  Weights on sync, x + small
            # tensors on scalar, so the critical (wq, xt) pair shares the
            # full HBM port instead of serializing on one queue.
            # weights split in halves so the completion semaphores unlock
            # the k=0,1 accumulation matmuls before the full tensor lands
            nc.sync.dma_start(out=wq_all[:, 0:2 * DIN],
                              in_=w_d["wq"][:, 0:2 * DIN])
            nc.sync.dma_start(out=wq_all[:, 2 * DIN:4 * DIN],
                              in_=w_d["wq"][:, 2 * DIN:4 * DIN])
            nc.sync.dma_start(out=wk_all[:, 0:2 * DIN],
                              in_=w_d["wk"][:, 0:2 * DIN])
            nc.sync.dma_start(out=wk_all[:, 2 * DIN:4 * DIN],
                              in_=w_d["wk"][:, 2 * DIN:4 * DIN])
            nc.sync.dma_start(out=wv_all, in_=w_d["wv"])
            # x split by column-chunk: each DMA covers cols [ch*CH, ch*CH+CH)
            # of all 4 k-slices (strided rows of 4x800B -- still big rows).
            # Triggers go on the gpsimd engine (idle early) so they don't
            # head-block the scalar (ACT) instruction stream.
            xa = xt_all.rearrange("p (k t) -> p k t", t=TOK)
            xd = xt_d.rearrange("p (k t) -> p k t", t=TOK)
            nc.gpsimd.dma_start(out=xa[:, :, 0:CH], in_=xd[:, :, 0:CH])
            nc.gpsimd.dma_start(out=xa[:, :, CH:TOK], in_=xd[:, :, CH:TOK])
            nc.gpsimd.dma_start(out=bqc, in_=bqc_d)
            nc.gpsimd.dma_start(out=bkc, in_=bkc_d)
            nc.gpsimd.dma_start(out=ma[0:5, :], in_=ma_d)
            nc.gpsimd.dma_start(out=ma[64:69, :], in_=ma_d)
            nc.gpsimd.dma_start(out=mb4[0:5, :], in_=mb4_d)
            nc.gpsimd.dma_start(out=mb4[64:69, :], in_=mb4_d)
            # broadcast the v-bias row to 100 partitions straight from DRAM
            # (stride-0 partition dim on the source AP)
            bv_src, _ = broadcast_tensor_aps(bvr_d, bvb[0:TT, :])
            nc.gpsimd.dma_start(out=bvb[0:TT, :], in_=bv_src)

            # ---- stage emitters ----
            def qk_group(w, bc, dst, ft, ch):
                fsl = slice(ft * 128, (ft + 1) * 128)
                csl = slice(ch * CH, (ch + 1) * CH)
                acc = ps.tile([128, CH], f32, name="acc", tag="acc", bufs=2)
                for k in range(4):
                    nc.tensor.matmul(acc[:], w[k][:, fsl], xt[k][:, csl],
                                     start=(k == 0), stop=(k == 3))
                # psum->sbuf copy + per-partition bias on DVE, keeping the
                # scalar engine free for the exp activations
                nc.vector.tensor_scalar_add(dst[ft][:, csl], acc[:],
                                            bc[:, ft:ft + 1])

            def v_tile(t):
                tsl = slice(t * TT, (t + 1) * TT)
                acc = ps.tile([TT, DIN], f32, name="vacc", tag="vacc", bufs=1)
                for k in range(4):
                    nc.tensor.matmul(acc[:], xt[k][:, tsl], wv[k][:],
                                     start=(k == 0), stop=(k == 3))
                vv = vt[t].rearrange("p (h c) -> p h c", c=D + 1)
                av = acc.rearrange("p (h c) -> p h c", c=D)
                bv = bvb[:TT, :].rearrange("p (h c) -> p h c", c=D)
                nc.vector.scalar_tensor_tensor(vv[:, :, :D], av, 0.0, bv,
                                               op0=ALU.add, op1=ALU.add)
                nc.vector.tensor_scalar_max(vv[:, :, :D], vv[:, :, :D], 0.0)
                nc.vector.memset(vv[:, :, D:D + 1], 1.0)

            def att_tile(t):
                tsl = slice(t * TT, (t + 1) * TT)
                # two banks: even heads (PE rows 0-63) / odd heads (64-127)
                stE = ps.tile([TT, 4 * TT], f32, name="stE", tag="st",
                              bufs=3)
                stO = ps.tile([TT, 4 * TT], f32, name="stO", tag="st",
                              bufs=3)
                nc.tensor.matmul(stE[:], ma[0:5, :], mb4[0:5, :],
                                 start=True, stop=False,
                                 skip_group_check=True)
                nc.tensor.matmul(stO[:], ma[64:69, :], mb4[64:69, :],
                                 start=True, stop=False,
                                 skip_group_check=True)
                for i in range(4):
                    c = slice(i * TT, (i + 1) * TT)
                    # head 2i: ft=i rows 0-63; head 2i+1: ft=i rows 64-127
                    nc.tensor.matmul(stE[:, c], kt_[i][0:64, tsl],
                                     qt[i][0:64, tsl],
                                     start=False, stop=(i == 3),
                                     skip_group_check=True)
                    nc.tensor.matmul(stO[:, c], kt_[i][64:128, tsl],
                                     qt[i][64:128, tsl],
                                     start=False, stop=(i == 3),
                                     skip_group_check=True)
                etE = sp.tile([TT, 4 * TT], pv_dt, name="etE", tag="et",
                              bufs=4)
                etO = sp.tile([TT, 4 * TT], pv_dt, name="etO", tag="et",
                              bufs=4)
                nc.scalar.activation(etE[:], stE[:], AF.Exp)
                nc.scalar.activation(etO[:], stO[:], AF.Exp)

                # PV: 2-bank psum [100, 2x512]; head h at bank h//4,
                # col (h%4)*65 (65 cols incl denominator)
                pv = ps.tile([TT, 1024], f32, name="pv", tag="pv", bufs=1)
                for h in range(H):
                    et = etE if h % 2 == 0 else etO
                    blk = h // 2
                    off = (h // 4) * 512 + (h % 4) * 65
                    nc.tensor.matmul(pv[:, off:off + 65],
                                     et[:, blk * TT:(blk + 1) * TT],
                                     vt[t][:, h * 65:(h + 1) * 65],
                                     start=True, stop=True,
                                     skip_group_check=True)
                pvb = pv.rearrange("p (b s) -> p b s", s=512)
                pvq = pvb[:, :, 0:4 * 65].rearrange("p b (q c) -> p b q c",
                                                    c=65)
                rc = sp.tile([TT, 8], f32, name="rc", tag="rc", bufs=4)
                rcv = rc.rearrange("p (b q c) -> p b q c", b=2, c=1)
                nc.vector.reciprocal(rcv, pvq[:, :, :, D:D + 1])
                ov = ot[t].rearrange("p (b q c) -> p b q c", b=2, c=D)
                i0, i1 = broadcast_tensor_aps(pvq[:, :, :, 0:D], rcv)
                nc.vector.tensor_tensor(ov, i0, i1, op=ALU.mult)
                nc.sync.dma_start(out=out_d[tsl, :], in_=ot[t][:])

            # ---- pipelined emission (matches DMA arrival order) ----
            for ch in range(2):
                for ft in range(4):
                    qk_group(wq, bqc, qt, ft, ch)
            for ch in range(2):
                for ft in range(4):
                    qk_group(wk, bkc, kt_, ft, ch)
            v_tile(0)
            v_tile(1)
            v_tile(2)
            att_tile(0)
            v_tile(3)
            att_tile(1)
            v_tile(4)
            att_tile(2)
            v_tile(5)
            att_tile(3)
            v_tile(6)
            att_tile(4)
            v_tile(7)
            att_tile(5)
            att_tile(6)
            att_tile(7)

    nc.compile()
    return nc


def _prep_inputs(x, Wq, bq, Wk, bk, Wv, bv, cfg):
    import ml_dtypes

    x = np.asarray(x, np.float32)
    Wq = np.asarray(Wq, np.float32)
    bq = np.asarray(bq, np.float32)
    Wk = np.asarray(Wk, np.float32)
    bk = np.asarray(bk, np.float32)
    Wv = np.asarray(Wv, np.float32)
    bv = np.asarray(bv, np.float32)

    scale = 1.0 / np.sqrt(np.float32(D))  # 1/8, exact
    wq_s = (Wq * scale).astype(np.float32)
    bq_s = (bq * scale).astype(np.float32)

    io_np = {"bf16": ml_dtypes.bfloat16,
             "f16": np.float16}.get(cfg["proj"], np.float32)
    mask_np = np.float16 if cfg["qk"] == "f16" else ml_dtypes.bfloat16
    xT = np.ascontiguousarray(x.transpose(0, 2, 1))  # [B, DIN, N]

    bqc = np.ascontiguousarray(bq_s.reshape(4, 128).T)
    bkc = np.ascontiguousarray(bk.reshape(4, 128).T)
    bvr = np.ascontiguousarray(bv[None, :])

    # rank-5 factors of the additive frame mask over one 100-token tile
    # (the kernel DMAs these 5 rows to partition bases 0 and 64)
    big = mask_np(NEGB)
    mA = np.zeros((5, TT), mask_np)
    mB = np.zeros((5, TT), mask_np)
    mA[0, :] = 1
    mB[0, :] = -big
    for f in range(4):
        mA[1 + f, f * JN:(f + 1) * JN] = 1
        mB[1 + f, f * JN:(f + 1) * JN] = big
    mB4 = np.ascontiguousarray(np.tile(mB, (1, 4)))

    def pack_w(w):
        # [512, 512] -> [128, 4*512]: k-slices side by side (2KB+ DMA rows)
        return np.ascontiguousarray(
            w.reshape(4, 128, DIN).transpose(1, 0, 2).reshape(128, 4 * DIN)
        ).astype(io_np)

    wq_p, wk_p, wv_p = pack_w(wq_s), pack_w(Wk), pack_w(Wv)

    in_maps = []
    for c in range(NCORES):
        b, fb = c // 4, c % 4
        xc = xT[b, :, fb * TOK:(fb + 1) * TOK]  # [512, 800]
        xt_p = np.ascontiguousarray(
            xc.reshape(4, 128, TOK).transpose(1, 0, 2).reshape(128, 4 * TOK)
        ).astype(io_np)
        in_maps.append({
            "xTp": xt_p,
            "wq": wq_p,
            "wk": wk_p,
            "wv": wv_p,
            "bqc": bqc, "bkc": bkc, "bvr": bvr,
            "mA": mA, "mB4": mB4,
        })
    return in_maps


def kernel(x, Wq, bq, Wk, bk, Wv, bv, att_heads=H, latent_dim=D,
           time_len=TL, joint_num=JN, **_):
    from concourse.bass_utils import run_bass_kernel_spmd

    cfg = tuple(sorted(CONFIG.items()))
    if cfg not in _CACHE:
        _CACHE[cfg] = _build(CONFIG)
    nc = _CACHE[cfg]

    in_maps = _prep_inputs(x, Wq, bq, Wk, bk, Wv, bv, CONFIG)
    res = run_bass_kernel_spmd(nc, in_maps, core_ids=list(range(NCORES)))
    global LAST_RESULT
    LAST_RESULT = res

    out = np.empty((B, N, DIN), np.float32)
    for c in range(NCORES):
        b, fb = c // 4, c % 4
        out[b, fb * TOK:(fb + 1) * TOK, :] = res.results[c]["out"]
    return out
